# revision 17
# baseline (speedup 1.0000x reference)
"""Trainium2 Bass kernel for nn_NodeInference (2-layer GAT + cosine head).

v4 design (SPMD, 8 cores, dst-node sharding, unified shard-order tables):
  Node n lives at shard row r(n) = core*6272 + block*128 + slot; the SAME row
  serves both GAT layers' tables, so one index set drives both edge phases.
  Table rows are 384 f16 (768B): [h 256xf16 | a_src f32 x nhead | pad].

  P1   dense-1 for OWN shard only -> cc1_in rows; a_dst(local) -> SBUF.
  AG1  AllGather cc1_in -> cc1_out (the layer-1 gather table).
  E1   per dst block (49): 17 gathered chunks of 128 edges (11 lo + 6 hi,
       Q7 dma_gather, 768B rows, int16 idx split at row 31360) + 1 contiguous
       self chunk from cc1_in. Batched DVE ops build the one-hot S, messages
       w*h, and edge weights w = exp(lrelu(a_s + a_d, 0.2)); a_d per edge via
       tiny S2^T @ adb matmuls where S2 = dma_start_transpose(S).
       PSUM accumulates S^T @ [msg | w] -> out1 -> leakyrelu(.,0.01) ->
       dense-2 -> cc2_in rows; a_dst2(local) -> SBUF.
  AG2  AllGather cc2_in -> cc2_out.
  E2   same for layer 2 (1 head); epilogue fuses the cosine head per block:
       out = h2 @ [g | g@mu-diag], nrm2 via square+reduce, staged to SBUF.
  F    one final pass: cos = num / max(nrm*|mu|, 1e-8) -> outD [6272, 8].
Host: balanced node->core (in-degree snake within out-degree groups) and
node->block packing (greedy FFD under lo/hi chunk caps).
"""

import sys
from dataclasses import dataclass
from contextlib import ExitStack

if "/opt/trn_rl_repo" not in sys.path:
    sys.path.insert(0, "/opt/trn_rl_repo")

import numpy as np

import concourse.bacc as bacc
import concourse.bass as bass
import concourse.mybir as mybir
import concourse.tile as tile

P = 128
IN = 256
H1 = 2            # layer-1 heads
HID = 256         # layer-1 out dim (2*128 concat)
OUT = 256         # layer-2 out dim
KH, MD = 8, 128   # cosine head
ROWW = 384        # f16 cols per table row (768B)
W = 8             # world size
NBLK = 49
CAP = NBLK * P    # 6272 rows per shard
NROWS = W * CAP   # 50176
LOSPLIT = 31360   # lo/hi table split (5 shards; both halves < 32768 rows)
AF = mybir.ActivationFunctionType
ALU = mybir.AluOpType
DT = mybir.dt


@dataclass
class CFG:
    N: int
    CPL: int   # lo chunks per block
    CPH: int   # hi chunks per block
    GAUG: int  # g_aug cols (1024 + 8)

    @property
    def CPG(self):  # gathered chunks
        return self.CPL + self.CPH

    @property
    def CPB(self):  # total chunks incl self
        return self.CPG + 1


def build_program(cfg: CFG):
    nc = bacc.Bacc("TRN2", target_bir_lowering=False, debug=False,
                   num_swdge_queues=4, dynamic_dma_scratch_size=65536)
    CPL, CPH, CPG, CPB = cfg.CPL, cfg.CPH, cfg.CPG, cfg.CPB
    AUG1, AUG2 = IN + 4, IN + 2
    GAUG = cfg.GAUG
    f16, f32, i16 = DT.float16, DT.float32, DT.int16
    bf16 = DT.bfloat16

    with tile.TileContext(nc) as tc, ExitStack() as stack:
        dram = stack.enter_context(
            tc.tile_pool(name="dram", bufs=1, space="DRAM"))

        def din(name, shape, dtype):
            return dram.tile(shape, dtype, kind="ExternalInput", name=name,
                             uniquify=False)

        xTi = din("xTi", [P, NBLK, 2, P], f16)
        w1s = din("w1s", [P, 2, AUG1], f16)
        w2s = din("w2s", [P, 2, AUG2], f16)
        gsd = din("gs", [P, 2, GAUG], f16)
        cmud = din("cmu", [P, KH], f32)       # |mu_k| host-broadcast
        b1d = din("b1b", [P, HID], f32)
        b2d = din("b2b", [P, OUT], f32)
        iotd = din("iota", [P, P], f16)       # iota[p, f] = f
        isrd = din("isrc", [P, NBLK * CPG * 8], i16)
        dstd = din("dstf", [P, NBLK * CPB], f16)
        s2hd = din("s2h", [P, NBLK * CPB * P], DT.bfloat16)
        outD = dram.tile([CAP, KH], f32, kind="ExternalOutput", name="outD",
                         uniquify=False)

        cc1_in = dram.tile([CAP, ROWW], f16, name="cc1_in")
        cc2_in = dram.tile([CAP, ROWW], f16, name="cc2_in")
        cc1_out = dram.tile([NROWS, ROWW], f16, name="cc1_out",
                            addr_space="Shared")
        cc2_out = dram.tile([NROWS, ROWW], f16, name="cc2_out",
                            addr_space="Shared")

        consts = stack.enter_context(tc.tile_pool(name="consts", bufs=1))
        w1_sb = consts.tile([P, 2, AUG1], f16)
        w2_sb = consts.tile([P, 2, AUG2], f16)
        g_sb = consts.tile([P, 2, GAUG], f16)
        cmu_sb = consts.tile([P, KH], f32)
        b1_sb = consts.tile([P, HID], f32)
        b2_sb = consts.tile([P, OUT], f32)
        iota_sb = consts.tile([P, P], f16)
        adL1 = consts.tile([P, NBLK, 2], bf16)   # a_dst of local nodes, L1
        adL2 = consts.tile([P, NBLK, 1], bf16)   # a_dst of local nodes, L2
        nrm2_st = consts.tile([P, NBLK, KH], f32)
        num_st = consts.tile([P, NBLK, KH], f32)
        c02 = consts.tile([P, 1], f32)
        c001 = consts.tile([P, 1], f32)
        nc.vector.memset(c02[:], 0.2)
        nc.vector.memset(c001[:], 0.01)

        for d, s in [(w1_sb, w1s), (w2_sb, w2s), (g_sb, gsd),
                     (cmu_sb, cmud), (b1_sb, b1d), (b2_sb, b2d),
                     (iota_sb, iotd)]:
            nc.sync.dma_start(d[:], s[:])

        # ---------------- P1: dense layer 1 (own shard only) ------------
        with tc.tile_pool(name="p1x", bufs=3) as p1x, \
             tc.tile_pool(name="p1ps", bufs=2, space="PSUM") as p1ps, \
             tc.tile_pool(name="p1row", bufs=3) as p1row:
            for t in range(NBLK):
                xt = p1x.tile([P, 2, P], f16, tag="xt")
                nc.sync.dma_start(xt[:], xTi[:, t, :, :])
                ps = p1ps.tile([P, AUG1], f32, tag="ps")
                for k in range(2):
                    nc.tensor.matmul(ps[:], lhsT=xt[:, k, :],
                                     rhs=w1_sb[:, k, :],
                                     start=(k == 0), stop=(k == 1))
                row = p1row.tile([P, IN + 4], f16, tag="row")
                nc.scalar.activation(row[:, 0:IN], ps[:, 0:IN], AF.Copy)
                rf = row[:].bitcast(f32)
                nc.vector.tensor_copy(rf[:, IN // 2:IN // 2 + 2],
                                      ps[:, IN:IN + 2])
                nc.vector.tensor_copy(adL1[:, t, :], ps[:, IN + 2:IN + 4])
                nc.sync.dma_start(cc1_in[t * P:(t + 1) * P, 0:IN + 4],
                                  row[:])

        nc.gpsimd.collective_compute(
            "AllGather", ALU.bypass, replica_groups=[list(range(W))],
            ins=[cc1_in[:]], outs=[cc1_out[:]])

        gq = [0]  # global Pool-DMA emission counter: sem lane = gq%8,
        # so queue gq%4 keeps each DMASW sem pinned to one SWDGE queue

        # ---- per-edge a_dst staging (filled by pre-pass in AG windows)
        ad_st1 = consts.tile([P, NBLK, CPB, 2], f32)
        ad_st2 = consts.tile([P, NBLK, CPB, 1], f32)

        # shared SBUF pools for both layers (avoids an SBUF reallocation
        # barrier at phase boundaries that would stall the AG windows)
        pi = stack.enter_context(tc.tile_pool(name="idx", bufs=3))
        pg = stack.enter_context(tc.tile_pool(name="gath", bufs=4))
        pe_ = stack.enter_context(tc.tile_pool(name="ew", bufs=3))
        pm = stack.enter_context(tc.tile_pool(name="msg", bufs=3))
        po = stack.enter_context(tc.tile_pool(name="epi", bufs=2))
        pA = stack.enter_context(tc.tile_pool(name="pA", bufs=3))
        # init gather buffers once: stale data must stay finite
        for ig in range(4):
            g0 = pg.tile([P, CPB, ROWW], f16, tag="gt")
            nc.vector.memset(g0[:], 0.0)

        # ---------------- pre-pass: S2^T @ adb -> staged a_dst ----------
        def phase_a(layer):
            adL, adst, nhead = ((adL1, ad_st1, 2) if layer == 1
                                else (adL2, ad_st2, 1))
            with tc.tile_pool(name=f"pAp{layer}", bufs=2,
                              space="PSUM") as pAp:
                for blk in range(NBLK):
                    S2 = pA.tile([P, CPB, P], bf16, tag="S2")
                    nc.sync.dma_start(
                        S2[:], s2hd[:, blk * CPB * P:(blk + 1) * CPB * P]
                        .rearrange("p (a b) -> p a b", a=CPB))
                    ad_ps = pAp.tile([P, CPB * nhead], f32, tag="adp")
                    for j in range(CPB):
                        nc.tensor.matmul(
                            ad_ps[:, j * nhead:(j + 1) * nhead],
                            lhsT=S2[:, j, :], rhs=adL[:, blk, :],
                            start=True, stop=True)
                    nc.vector.tensor_copy(
                        adst[:, blk].rearrange("p a b -> p (a b)"), ad_ps[:])

        # ---------------- main pass ------------------------------------
        def edge_phase(layer):
            if layer == 1:
                table, self_tab = cc1_out, cc1_in
                nhead, MSGC, adst, b_sb = 2, HID + 2, ad_st1, b1_sb
            else:
                table, self_tab = cc2_out, cc2_in
                nhead, MSGC, adst, b_sb = 1, OUT + 1, ad_st2, b2_sb
            tab_lo = table[0:LOSPLIT, :]
            tab_hi = table[LOSPLIT:NROWS, :]

            es = ExitStack()
            pp = es.enter_context(
                tc.tile_pool(name=f"bps{layer}", bufs=3, space="PSUM"))
            ph = es.enter_context(
                tc.tile_pool(name=f"hps{layer}", bufs=2, space="PSUM"))
            pn = es.enter_context(
                tc.tile_pool(name=f"nps{layer}", bufs=1, space="PSUM"))

            # S one-hots built one block ahead of consumption
            S_t = {}

            def prep_S(b):
                dstf = pi.tile([P, CPB], f16, tag="dstf")
                nc.sync.dma_start(dstf[:], dstd[:, b * CPB:(b + 1) * CPB])
                S = pm.tile([P, CPB, P], bf16, tag="S")
                nc.vector.tensor_tensor(
                    S[:],
                    iota_sb[:].unsqueeze(1).to_broadcast([P, CPB, P]),
                    dstf[:].unsqueeze(2).to_broadcast([P, CPB, P]),
                    op=ALU.is_equal)
                S_t[b] = S

            prep_S(0)
            for blk in range(NBLK):
                isrc = pi.tile([P, CPG * 8], i16, tag="isrc")
                nc.sync.dma_start(isrc[:],
                                  isrd[:, blk * CPG * 8:(blk + 1) * CPG * 8])

                gt = pg.tile([P, CPB, ROWW], f16, tag="gt")
                for c0 in range(0, CPL, 8):
                    c1 = min(c0 + 8, CPL)
                    nc.gpsimd.dma_gather(
                        gt[:, c0:c1, :], tab_lo,
                        isrc[:, c0 * 8:c1 * 8],
                        (c1 - c0) * P, (c1 - c0) * P, ROWW,
                        queue_num=gq[0] % 4)
                    gq[0] += 1
                for c0 in range(CPL, CPG, 8):
                    c1 = min(c0 + 8, CPG)
                    nc.gpsimd.dma_gather(
                        gt[:, c0:c1, :], tab_hi,
                        isrc[:, c0 * 8:c1 * 8],
                        (c1 - c0) * P, (c1 - c0) * P, ROWW,
                        queue_num=gq[0] % 4)
                    gq[0] += 1
                # self chunk: contiguous rows of own shard
                nc.sync.dma_start(gt[:, CPG, 0:IN + 2 * nhead],
                                  self_tab[blk * P:(blk + 1) * P,
                                           0:IN + 2 * nhead])

                if blk + 1 < NBLK:
                    prep_S(blk + 1)
                S_all = S_t.pop(blk)

                # ew = exp(lrelu(a_s + a_d, 0.2))
                gtf = gt[:].bitcast(f32)
                as_v = gtf[:, :, IN // 2:IN // 2 + nhead]
                ew_t = pe_.tile([P, CPB, 2], f32, tag="ew")
                ew = ew_t[:, :, 0:nhead]
                nc.vector.tensor_tensor(ew, as_v, adst[:, blk],
                                        op=ALU.add)
                ew2_t = pe_.tile([P, CPB, 2], f32, tag="ew2")
                ew2 = ew2_t[:, :, 0:nhead]
                nc.vector.tensor_tensor(
                    ew2, ew,
                    c02[:].unsqueeze(2).to_broadcast([P, CPB, nhead]),
                    op=ALU.mult)
                nc.vector.tensor_tensor(ew, ew2, ew, op=ALU.max)
                wv_t = pe_.tile([P, CPB, 2], bf16, tag="wv")
                wv = wv_t[:, :, 0:nhead]
                nc.scalar.activation(wv, ew, AF.Exp)

                # msg = [w*h | w], two halves so bp matmuls start early
                msg_t = pm.tile([P, CPB, HID + 2], bf16, tag="msg")
                msg = msg_t[:, :, 0:MSGC]
                bp_t = pp.tile([P, HID + 2], f32, tag="bp")
                bp = bp_t[:, 0:MSGC]
                HSPL = CPB // 2
                for h0, h1 in ((0, HSPL), (HSPL, CPB)):
                    nc.vector.tensor_tensor(
                        msg[:, h0:h1, 0:IN].rearrange(
                            "p a (h f) -> p a h f", h=nhead),
                        gt[:, h0:h1, 0:IN].rearrange(
                            "p a (h f) -> p a h f", h=nhead),
                        wv[:, h0:h1, :].unsqueeze(3).to_broadcast(
                            [P, h1 - h0, nhead, IN // nhead]),
                        op=ALU.mult)
                    nc.vector.tensor_copy(msg[:, h0:h1, IN:IN + nhead],
                                          wv[:, h0:h1, :])
                    for j in range(h0, h1):
                        nc.tensor.matmul(bp, lhsT=S_all[:, j, :],
                                         rhs=msg[:, j, :],
                                         start=(j == 0), stop=(j == CPB - 1))
                # ---- epilogue
                rec = po.tile([P, nhead], f32, tag="rec")
                nc.vector.reciprocal(rec[:], bp[:, IN:IN + nhead])
                ti = po.tile([P, IN], f32, tag="ti")
                nc.vector.tensor_tensor(
                    ti[:].rearrange("p (h f) -> p h f", h=nhead),
                    bp[:, 0:IN].rearrange("p (h f) -> p h f", h=nhead),
                    rec[:].unsqueeze(2).to_broadcast(
                        [P, nhead, IN // nhead]),
                    op=ALU.mult)
                nc.vector.tensor_tensor(ti[:], ti[:], b_sb[:], op=ALU.add)
                tif = po.tile([P, IN], f16, tag="tif")
                if layer == 1:
                    # leaky_relu 0.01 then cast
                    tl = po.tile([P, IN], f32, tag="tl")
                    nc.vector.tensor_tensor(
                        tl[:], ti[:], c001[:].to_broadcast([P, IN]),
                        op=ALU.mult)
                    nc.vector.tensor_tensor(tif[:], tl[:], ti[:], op=ALU.max)
                else:
                    nc.scalar.activation(tif[:], ti[:], AF.Copy)
                hT = po.tile([P, 2, P], f16, tag="hT")
                nc.sync.dma_start_transpose(hT[:], tif[:])

                if layer == 1:
                    hp = ph.tile([P, AUG2], f32, tag="hp")
                    for k in range(2):
                        nc.tensor.matmul(hp[:], lhsT=hT[:, k, :],
                                         rhs=w2_sb[:, k, :],
                                         start=(k == 0), stop=(k == 1))
                    row2 = po.tile([P, IN + 2], f16, tag="row2")
                    nc.scalar.activation(row2[:, 0:IN], hp[:, 0:IN], AF.Copy)
                    r2f = row2[:].bitcast(f32)
                    nc.vector.tensor_copy(r2f[:, IN // 2:IN // 2 + 1],
                                          hp[:, IN:IN + 1])
                    nc.vector.tensor_copy(adL2[:, blk, :],
                                          hp[:, IN + 1:IN + 2])
                    nc.sync.dma_start(cc2_in[blk * P:(blk + 1) * P, 0:IN + 2],
                                      row2[:])
                else:
                    # fused cosine head: out = h2 @ [g | gm]
                    op_ = ph.tile([P, KH * MD], f32, tag="op")
                    for k in range(2):
                        nc.tensor.matmul(op_[:, 0:512], lhsT=hT[:, k, :],
                                         rhs=g_sb[:, k, 0:512],
                                         start=(k == 0), stop=(k == 1))
                    for k in range(2):
                        nc.tensor.matmul(op_[:, 512:KH * MD],
                                         lhsT=hT[:, k, :],
                                         rhs=g_sb[:, k, 512:KH * MD],
                                         start=(k == 0), stop=(k == 1))
                    num_ps = pn.tile([P, KH], f32, tag="nm")
                    num_ps = num_ps[:]
                    for k in range(2):
                        nc.tensor.matmul(num_ps, lhsT=hT[:, k, :],
                                         rhs=g_sb[:, k, KH * MD:GAUG],
                                         start=(k == 0), stop=(k == 1))
                    for k in range(KH):
                        sqs = po.tile([P, MD], f16, tag="sqs")
                        nc.scalar.activation(
                            sqs[:], op_[:, k * MD:(k + 1) * MD], AF.Square,
                            accum_out=nrm2_st[:, blk, k:k + 1])
                    nc.vector.tensor_copy(num_st[:, blk, :], num_ps)

            es.close()

        phase_a(1)
        edge_phase(1)
        phase_a(2)
        nc.gpsimd.collective_compute(
            "AllGather", ALU.bypass, replica_groups=[list(range(W))],
            ins=[cc2_in[:]], outs=[cc2_out[:]])
        edge_phase(2)

        # ---------------- final cosine ----------------------------------
        with tc.tile_pool(name="fin", bufs=1) as fin:
            nrm = fin.tile([P, NBLK, KH], f32)
            nc.scalar.activation(nrm[:], nrm2_st[:], AF.Sqrt)
            nc.vector.tensor_tensor(
                nrm[:], nrm[:],
                cmu_sb[:].unsqueeze(1).to_broadcast([P, NBLK, KH]),
                op=ALU.mult)
            nc.vector.tensor_scalar(out=nrm[:], in0=nrm[:], scalar1=1e-8,
                                    scalar2=None, op0=ALU.max)
            rcp = fin.tile([P, NBLK, KH], f32)
            nc.vector.reciprocal(rcp[:], nrm[:])
            res = fin.tile([P, NBLK, KH], f32)
            nc.vector.tensor_tensor(res[:], num_st[:], rcp[:], op=ALU.mult)
            nc.sync.dma_start(
                outD[:].rearrange("(b p) k -> p b k", p=P), res[:])

    nc.compile()
    return nc


# ===================== host-side preparation ============================

def _wrap16(flat):
    """flat idx [n] -> wrapped int16 [128, n//16] (8 Q7-core replicas)."""
    n = len(flat)
    out = np.zeros((P, n // 16), np.int16)
    cols = np.arange(n) // 16
    rows = np.arange(n) % 16
    for r in range(8):
        out[r * 16 + rows, cols] = flat
    return out


def prep_host(x, edge_index, W1, a_src1, a_dst1, b1, W2, a_src2, a_dst2, b2,
              g, mu):
    x = np.asarray(x, np.float32)
    N = x.shape[0]
    src = np.asarray(edge_index[0], np.int64)
    dst = np.asarray(edge_index[1], np.int64)
    E = len(src)

    od = np.bincount(src, minlength=N)
    idg = np.bincount(dst, minlength=N)

    # node -> core: top-5/8 by out-degree to cores 0-4 (lo table half),
    # in-degree snake within each group for compute balance.
    PERCORE = N // W
    order_od = np.argsort(-od, kind="stable")
    groupA = order_od[:5 * PERCORE]
    groupB = order_od[5 * PERCORE:]
    core_of = np.empty(N, np.int32)
    for base, grp in ((0, groupA), (5, groupB)):
        ncg = len(grp) // PERCORE
        gs = grp[np.argsort(-idg[grp], kind="stable")]
        pat = np.concatenate([np.arange(ncg), np.arange(ncg)[::-1]])
        asn = np.tile(pat, (len(gs) + 2 * ncg - 1) // (2 * ncg))[:len(gs)]
        core_of[gs] = base + asn

    # per-edge lo flag: src in cores 0-4
    src_lo = core_of[src] <= 4

    # per-node lo/hi in-degree
    idl = np.bincount(dst[src_lo], minlength=N)
    idh = idg - idl

    # per-core block packing under (lo, hi, count) caps
    loS = max(idl[core_of == c].sum() for c in range(W))
    hiS = max(idh[core_of == c].sum() for c in range(W))
    cands = []
    for tot in range(16, 24):
        for cl in range(1, tot):
            ch = tot - cl
            if loS <= 0.975 * NBLK * cl * P and hiS <= 0.975 * NBLK * ch * P:
                cands.append((cl, ch))
    ci = 0
    while True:
        CPL, CPH = cands[ci]
        capL, capH = CPL * P, CPH * P
        blk_of = np.full(N, -1, np.int32)
        slot_of = np.full(N, -1, np.int32)
        ok = True
        for c in range(W):
            nodes = np.where(core_of == c)[0]
            nodes = nodes[np.argsort(-(idl[nodes] + idh[nodes]),
                                     kind="stable")]
            bl = np.zeros(NBLK, np.int64)
            bh = np.zeros(NBLK, np.int64)
            bn = np.zeros(NBLK, np.int64)
            for n in nodes:
                load = np.maximum((bl + idl[n]) / capL, (bh + idh[n]) / capH)
                load[bn >= P] = 10.0
                load[(bl + idl[n]) > capL] = 10.0
                load[(bh + idh[n]) > capH] = 10.0
                b = int(np.argmin(load))
                if load[b] >= 10.0:
                    ok = False
                    break
                blk_of[n] = b
                slot_of[n] = bn[b]
                bl[b] += idl[n]
                bh[b] += idh[n]
                bn[b] += 1
            if not ok:
                break
        if ok:
            break
        ci += 1

    cfg = CFG(N=N, CPL=CPL, CPH=CPH, GAUG=KH * MD + KH)
    CPG, CPB = cfg.CPG, cfg.CPB

    row_of = core_of.astype(np.int64) * CAP + blk_of * P + slot_of

    # group edges by (core, block), lo first then hi, sorted by src row
    gkey = core_of[dst].astype(np.int64) * NBLK + blk_of[dst]
    skey = gkey * 2 + (~src_lo)
    order = np.argsort(skey * NROWS + row_of[src], kind="stable")
    esrc_r = row_of[src][order]
    edst_l = slot_of[dst][order].astype(np.int64)
    eslo = src_lo[order]
    ekey = gkey[order]
    starts = np.zeros(W * NBLK + 1, np.int64)
    cnts = np.bincount(ekey, minlength=W * NBLK)
    starts[1:] = np.cumsum(cnts)
    lo_cnt = np.bincount(ekey[eslo], minlength=W * NBLK)

    import ml_dtypes
    isrc_all = []
    dstf_all = []
    s2h_all = []
    for c in range(W):
        isrc = np.full((P, NBLK * CPG * 8), -1, np.int16)
        dstf = np.full((P, NBLK * CPB), -1.0, np.float16)
        dint = np.full((P, NBLK * CPB), -1, np.int64)
        for b in range(NBLK):
            gid = c * NBLK + b
            s0, s1 = starts[gid], starts[gid + 1]
            nlo = int(lo_cnt[gid])
            rows_ = esrc_r[s0:s1]
            dl = edst_l[s0:s1]
            fl = np.zeros(CPG * P, np.int64)   # pad: fetch row 0 (valid)
            fd = np.full(CPG * P, -1, np.int64)
            fl[:nlo] = rows_[:nlo]
            fd[:nlo] = dl[:nlo]
            nh = (s1 - s0) - nlo
            fl[CPL * P:CPL * P + nh] = rows_[nlo:] - LOSPLIT
            fd[CPL * P:CPL * P + nh] = dl[nlo:]
            isrc[:, b * CPG * 8:(b + 1) * CPG * 8] = _wrap16(fl)
            dcol = fd.reshape(CPG, P).T.astype(np.float16)
            dstf[:, b * CPB:b * CPB + CPG] = dcol
            dstf[:, b * CPB + CPG] = np.arange(P, dtype=np.float16)
            dint[:, b * CPB:b * CPB + CPG] = fd.reshape(CPG, P).T
            dint[:, b * CPB + CPG] = np.arange(P)
        # transposed one-hot: S2h[d, (b j e)] = 1 iff dst(e-slot, b, j) == d
        s2 = np.zeros((P, NBLK * CPB * P), ml_dtypes.bfloat16)
        ee, bj = np.nonzero(dint >= 0)
        s2[dint[ee, bj], bj * P + ee] = 1
        isrc_all.append(isrc)
        dstf_all.append(dstf)
        s2h_all.append(s2)

    # weights
    W1 = np.asarray(W1, np.float32)
    W2 = np.asarray(W2, np.float32)
    W1r = W1.reshape(H1, MD, IN)
    Ps1 = np.einsum("hdi,hd->ih", W1r, np.asarray(a_src1, np.float32))
    Pd1 = np.einsum("hdi,hd->ih", W1r, np.asarray(a_dst1, np.float32))
    W1aug = np.concatenate([W1.T, Ps1, Pd1], axis=1)
    Ps2 = W2.T @ np.asarray(a_src2, np.float32)[0][:, None]
    Pd2 = W2.T @ np.asarray(a_dst2, np.float32)[0][:, None]
    W2aug = np.concatenate([W2.T, Ps2, Pd2], axis=1)
    AUG1, AUG2 = IN + 4, IN + 2
    w1s = W1aug.reshape(2, P, AUG1).transpose(1, 0, 2).astype(np.float16)
    w2s = W2aug.reshape(2, P, AUG2).transpose(1, 0, 2).astype(np.float16)

    gm = np.asarray(g, np.float32)
    mu = np.asarray(mu, np.float32)
    gmu = np.einsum("fkm,km->fk", gm.reshape(IN, KH, MD), mu)  # [256, 8]
    g_aug = np.concatenate([gm, gmu], axis=1)                  # [256, 1032]
    gsd = g_aug.reshape(2, P, cfg.GAUG).transpose(1, 0, 2).astype(np.float16)
    cmu = np.broadcast_to(np.linalg.norm(mu, axis=1), (P, KH)).astype(
        np.float32).copy()
    b1b = np.broadcast_to(np.asarray(b1, np.float32), (P, HID)).copy()
    b2b = np.broadcast_to(np.asarray(b2, np.float32), (P, OUT)).copy()
    iota = np.broadcast_to(np.arange(P, dtype=np.float16), (P, P)).copy()

    shared = dict(w1s=w1s, w2s=w2s, gs=gsd, cmu=cmu, b1b=b1b, b2b=b2b,
                  iota=iota)
    in_maps = []
    for c in range(W):
        nodes = np.where(core_of == c)[0]
        xp = np.zeros((CAP, IN), np.float32)
        xp[blk_of[nodes] * P + slot_of[nodes]] = x[nodes]
        xTc = xp.reshape(NBLK, P, 2, P).transpose(3, 0, 2, 1).astype(
            np.float16)
        m = dict(shared)
        m.update(xTi=xTc, isrc=isrc_all[c], dstf=dstf_all[c],
                 s2h=s2h_all[c])
        in_maps.append(m)
    return cfg, in_maps, row_of


def assemble(outs, row_of, N):
    full = np.zeros((N, KH), np.float32)
    core = row_of // CAP
    rrow = row_of % CAP
    for c in range(W):
        sel = core == c
        full[sel] = outs[c]["outD"][rrow[sel]]
    return full


_CACHE = {}


def kernel(**inputs):
    cfg, in_maps, row_of = prep_host(**inputs)
    key = (cfg.N, cfg.CPL, cfg.CPH)
    if key not in _CACHE:
        _CACHE[key] = build_program(cfg)
    nc = _CACHE[key]
    from concourse.bass_utils import run_bass_kernel_spmd
    res = run_bass_kernel_spmd(nc, in_maps, core_ids=list(range(W)))
    return assemble(res.results, row_of, cfg.N)


# revision 18
# speedup vs baseline: 1.1022x; 1.1022x over previous
"""Trainium2 Bass kernel for nn_NodeInference (2-layer GAT + cosine head).

v4 design (SPMD, 8 cores, dst-node sharding, unified shard-order tables):
  Node n lives at shard row r(n) = core*6272 + block*128 + slot; the SAME row
  serves both GAT layers' tables, so one index set drives both edge phases.
  Table rows are 384 f16 (768B): [h 256xf16 | a_src f32 x nhead | pad].

  P1   dense-1 for OWN shard only -> cc1_in rows; a_dst(local) -> SBUF.
  AG1  AllGather cc1_in -> cc1_out (the layer-1 gather table).
  E1   per dst block (49): 17 gathered chunks of 128 edges (11 lo + 6 hi,
       Q7 dma_gather, 768B rows, int16 idx split at row 31360) + 1 contiguous
       self chunk from cc1_in. Batched DVE ops build the one-hot S, messages
       w*h, and edge weights w = exp(lrelu(a_s + a_d, 0.2)); a_d per edge via
       tiny S2^T @ adb matmuls where S2 = dma_start_transpose(S).
       PSUM accumulates S^T @ [msg | w] -> out1 -> leakyrelu(.,0.01) ->
       dense-2 -> cc2_in rows; a_dst2(local) -> SBUF.
  AG2  AllGather cc2_in -> cc2_out.
  E2   same for layer 2 (1 head); epilogue fuses the cosine head per block:
       out = h2 @ [g | g@mu-diag], nrm2 via square+reduce, staged to SBUF.
  F    one final pass: cos = num / max(nrm*|mu|, 1e-8) -> outD [6272, 8].
Host: balanced node->core (in-degree snake within out-degree groups) and
node->block packing (greedy FFD under lo/hi chunk caps).
"""

import sys
from dataclasses import dataclass
from contextlib import ExitStack

if "/opt/trn_rl_repo" not in sys.path:
    sys.path.insert(0, "/opt/trn_rl_repo")

import numpy as np

import concourse.bacc as bacc
import concourse.bass as bass
import concourse.mybir as mybir
import concourse.tile as tile

P = 128
IN = 256
H1 = 2            # layer-1 heads
HID = 256         # layer-1 out dim (2*128 concat)
OUT = 256         # layer-2 out dim
KH, MD = 8, 128   # cosine head
ROWW = 384        # f16 cols per table row (768B)
W = 8             # world size
NBLK = 49
CAP = NBLK * P    # 6272 rows per shard
NROWS = W * CAP   # 50176
LOSPLIT = 31360   # lo/hi table split (5 shards; both halves < 32768 rows)
AF = mybir.ActivationFunctionType
ALU = mybir.AluOpType
DT = mybir.dt


@dataclass
class CFG:
    N: int
    CPL: int   # lo chunks per block
    CPH: int   # hi chunks per block
    GAUG: int  # g_aug cols (1024 + 8)

    @property
    def CPG(self):  # gathered chunks
        return self.CPL + self.CPH

    @property
    def CPB(self):  # total chunks incl self
        return self.CPG + 1


def build_program(cfg: CFG):
    nc = bacc.Bacc("TRN2", target_bir_lowering=False, debug=False,
                   num_swdge_queues=4, dynamic_dma_scratch_size=65536)
    CPL, CPH, CPG, CPB = cfg.CPL, cfg.CPH, cfg.CPG, cfg.CPB
    AUG1, AUG2 = IN + 4, IN + 2
    GAUG = cfg.GAUG
    f16, f32, i16 = DT.float16, DT.float32, DT.int16
    bf16 = DT.bfloat16

    with tile.TileContext(nc) as tc, ExitStack() as stack:
        dram = stack.enter_context(
            tc.tile_pool(name="dram", bufs=1, space="DRAM"))

        def din(name, shape, dtype):
            return dram.tile(shape, dtype, kind="ExternalInput", name=name,
                             uniquify=False)

        xTi = din("xTi", [P, NBLK, 2, P], f16)
        w1s = din("w1s", [P, 2, AUG1], f16)
        w2s = din("w2s", [P, 2, AUG2], f16)
        gsd = din("gs", [P, 2, GAUG], f16)
        cmud = din("cmu", [P, KH], f32)       # |mu_k| host-broadcast
        b1d = din("b1b", [P, HID], f32)
        b2d = din("b2b", [P, OUT], f32)
        iotd = din("iota", [P, P], f16)       # iota[p, f] = f
        isrd = din("isrc", [P, NBLK * CPG * 8], i16)
        dstd = din("dstf", [P, NBLK * CPB], f16)
        s2hd = din("s2h", [P, NBLK * CPB * P], DT.bfloat16)
        outD = dram.tile([CAP, KH], f32, kind="ExternalOutput", name="outD",
                         uniquify=False)

        cc1_in = dram.tile([CAP, ROWW], f16, name="cc1_in")
        cc2_in = dram.tile([CAP, ROWW], f16, name="cc2_in")
        cc1_out = dram.tile([NROWS, ROWW], f16, name="cc1_out",
                            addr_space="Shared")
        cc2_out = dram.tile([NROWS, ROWW], f16, name="cc2_out",
                            addr_space="Shared")

        consts = stack.enter_context(tc.tile_pool(name="consts", bufs=1))
        w1_sb = consts.tile([P, 2, AUG1], f16)
        w2_sb = consts.tile([P, 2, AUG2], f16)
        g_sb = consts.tile([P, 2, GAUG], f16)
        cmu_sb = consts.tile([P, KH], f32)
        b1_sb = consts.tile([P, HID], f32)
        b2_sb = consts.tile([P, OUT], f32)
        iota_sb = consts.tile([P, P], f16)
        adL1 = consts.tile([P, NBLK, 2], bf16)   # a_dst of local nodes, L1
        adL2 = consts.tile([P, NBLK, 1], bf16)   # a_dst of local nodes, L2
        nrm2_st = consts.tile([P, NBLK, KH], f32)
        num_st = consts.tile([P, NBLK, KH], f32)
        c02 = consts.tile([P, 1], f32)
        c001 = consts.tile([P, 1], f32)
        nc.vector.memset(c02[:], 0.2)
        nc.vector.memset(c001[:], 0.01)

        for d, s in [(w1_sb, w1s), (w2_sb, w2s), (g_sb, gsd),
                     (cmu_sb, cmud), (b1_sb, b1d), (b2_sb, b2d),
                     (iota_sb, iotd)]:
            nc.sync.dma_start(d[:], s[:])

        # ---------------- P1: dense layer 1 (own shard only) ------------
        with tc.tile_pool(name="p1x", bufs=3) as p1x, \
             tc.tile_pool(name="p1ps", bufs=2, space="PSUM") as p1ps, \
             tc.tile_pool(name="p1row", bufs=3) as p1row:
            for t in range(NBLK):
                xt = p1x.tile([P, 2, P], f16, tag="xt")
                nc.sync.dma_start(xt[:], xTi[:, t, :, :])
                ps = p1ps.tile([P, AUG1], f32, tag="ps")
                for k in range(2):
                    nc.tensor.matmul(ps[:], lhsT=xt[:, k, :],
                                     rhs=w1_sb[:, k, :],
                                     start=(k == 0), stop=(k == 1))
                row = p1row.tile([P, IN + 4], f16, tag="row")
                nc.scalar.activation(row[:, 0:IN], ps[:, 0:IN], AF.Copy)
                rf = row[:].bitcast(f32)
                nc.vector.tensor_copy(rf[:, IN // 2:IN // 2 + 2],
                                      ps[:, IN:IN + 2])
                nc.vector.tensor_copy(adL1[:, t, :], ps[:, IN + 2:IN + 4])
                nc.sync.dma_start(cc1_in[t * P:(t + 1) * P, 0:IN + 4],
                                  row[:])

        nc.gpsimd.collective_compute(
            "AllGather", ALU.bypass, replica_groups=[list(range(W))],
            ins=[cc1_in[:]], outs=[cc1_out[:]])

        gq = [0]  # global Pool-DMA emission counter: sem lane = gq%8,
        # so queue gq%4 keeps each DMASW sem pinned to one SWDGE queue

        # ---- per-edge a_dst staging (filled by pre-pass in AG windows)
        ad_st1 = consts.tile([P, NBLK, CPB, 2], f32)
        ad_st2 = consts.tile([P, NBLK, CPB, 1], f32)

        # shared SBUF pools for both layers (avoids an SBUF reallocation
        # barrier at phase boundaries that would stall the AG windows)
        pi = stack.enter_context(tc.tile_pool(name="idx", bufs=3))
        pg = stack.enter_context(tc.tile_pool(name="gath", bufs=4))
        pe_ = stack.enter_context(tc.tile_pool(name="ew", bufs=3))
        pm = stack.enter_context(tc.tile_pool(name="msg", bufs=3))
        po = stack.enter_context(tc.tile_pool(name="epi", bufs=2))
        pA = stack.enter_context(tc.tile_pool(name="pA", bufs=3))
        # init gather buffers once: stale data must stay finite
        for ig in range(4):
            g0 = pg.tile([P, CPB, ROWW], f16, tag="gt")
            nc.vector.memset(g0[:], 0.0)

        # ---------------- pre-pass: S2^T @ adb -> staged a_dst ----------
        def phase_a(layer):
            adL, adst, nhead = ((adL1, ad_st1, 2) if layer == 1
                                else (adL2, ad_st2, 1))
            with tc.tile_pool(name=f"pAp{layer}", bufs=2,
                              space="PSUM") as pAp:
                for blk in range(NBLK):
                    S2 = pA.tile([P, CPB, P], bf16, tag="S2")
                    nc.sync.dma_start(
                        S2[:], s2hd[:, blk * CPB * P:(blk + 1) * CPB * P]
                        .rearrange("p (a b) -> p a b", a=CPB))
                    ad_ps = pAp.tile([P, CPB * nhead], f32, tag="adp")
                    for j in range(CPB):
                        nc.tensor.matmul(
                            ad_ps[:, j * nhead:(j + 1) * nhead],
                            lhsT=S2[:, j, :], rhs=adL[:, blk, :],
                            start=True, stop=True)
                    nc.vector.tensor_copy(
                        adst[:, blk].rearrange("p a b -> p (a b)"), ad_ps[:])

        # ---------------- main pass ------------------------------------
        def edge_phase(layer):
            if layer == 1:
                table, self_tab = cc1_out, cc1_in
                nhead, MSGC, adst, b_sb = 2, HID + 2, ad_st1, b1_sb
            else:
                table, self_tab = cc2_out, cc2_in
                nhead, MSGC, adst, b_sb = 1, OUT + 1, ad_st2, b2_sb
            tab_lo = table[0:LOSPLIT, :]
            tab_hi = table[LOSPLIT:NROWS, :]

            es = ExitStack()
            pp = es.enter_context(
                tc.tile_pool(name=f"bps{layer}", bufs=2, space="PSUM"))
            ph = es.enter_context(
                tc.tile_pool(name=f"hps{layer}", bufs=2, space="PSUM"))
            pn = es.enter_context(
                tc.tile_pool(name=f"nps{layer}", bufs=2, space="PSUM"))

            # S one-hots built one block ahead of consumption
            S_t = {}

            def prep_S(b):
                dstf = pi.tile([P, CPB], f16, tag="dstf")
                nc.sync.dma_start(dstf[:], dstd[:, b * CPB:(b + 1) * CPB])
                S = pm.tile([P, CPB, P], bf16, tag="S")
                nc.vector.tensor_tensor(
                    S[:],
                    iota_sb[:].unsqueeze(1).to_broadcast([P, CPB, P]),
                    dstf[:].unsqueeze(2).to_broadcast([P, CPB, P]),
                    op=ALU.is_equal)
                S_t[b] = S

            prep_S(0)
            for blk in range(NBLK):
                isrc = pi.tile([P, CPG * 8], i16, tag="isrc")
                nc.sync.dma_start(isrc[:],
                                  isrd[:, blk * CPG * 8:(blk + 1) * CPG * 8])

                gt = pg.tile([P, CPB, ROWW], f16, tag="gt")
                for c0 in range(0, CPL, 8):
                    c1 = min(c0 + 8, CPL)
                    nc.gpsimd.dma_gather(
                        gt[:, c0:c1, :], tab_lo,
                        isrc[:, c0 * 8:c1 * 8],
                        (c1 - c0) * P, (c1 - c0) * P, ROWW,
                        queue_num=gq[0] % 4)
                    gq[0] += 1
                for c0 in range(CPL, CPG, 8):
                    c1 = min(c0 + 8, CPG)
                    nc.gpsimd.dma_gather(
                        gt[:, c0:c1, :], tab_hi,
                        isrc[:, c0 * 8:c1 * 8],
                        (c1 - c0) * P, (c1 - c0) * P, ROWW,
                        queue_num=gq[0] % 4)
                    gq[0] += 1
                # self chunk: contiguous rows of own shard
                nc.sync.dma_start(gt[:, CPG, 0:IN + 2 * nhead],
                                  self_tab[blk * P:(blk + 1) * P,
                                           0:IN + 2 * nhead])

                if blk + 1 < NBLK:
                    prep_S(blk + 1)
                S_all = S_t.pop(blk)

                # ew = exp(lrelu(a_s + a_d, 0.2))
                gtf = gt[:].bitcast(f32)
                as_v = gtf[:, :, IN // 2:IN // 2 + nhead]
                ew_t = pe_.tile([P, CPB, 2], f32, tag="ew")
                ew = ew_t[:, :, 0:nhead]
                nc.vector.tensor_tensor(ew, as_v, adst[:, blk],
                                        op=ALU.add)
                ew2_t = pe_.tile([P, CPB, 2], f32, tag="ew2")
                ew2 = ew2_t[:, :, 0:nhead]
                nc.vector.tensor_tensor(
                    ew2, ew,
                    c02[:].unsqueeze(2).to_broadcast([P, CPB, nhead]),
                    op=ALU.mult)
                nc.vector.tensor_tensor(ew, ew2, ew, op=ALU.max)
                wv_t = pe_.tile([P, CPB, 2], bf16, tag="wv")
                wv = wv_t[:, :, 0:nhead]
                nc.scalar.activation(wv, ew, AF.Exp)

                # msg = [w*h | w], two halves so bp matmuls start early
                msg_t = pm.tile([P, CPB, HID + 2], bf16, tag="msg")
                msg = msg_t[:, :, 0:MSGC]
                bp_t = pp.tile([P, HID + 2], f32, tag="bp")
                bp = bp_t[:, 0:MSGC]
                HSPL = CPB // 2
                for h0, h1 in ((0, HSPL), (HSPL, CPB)):
                    nc.vector.tensor_tensor(
                        msg[:, h0:h1, 0:IN].rearrange(
                            "p a (h f) -> p a h f", h=nhead),
                        gt[:, h0:h1, 0:IN].rearrange(
                            "p a (h f) -> p a h f", h=nhead),
                        wv[:, h0:h1, :].unsqueeze(3).to_broadcast(
                            [P, h1 - h0, nhead, IN // nhead]),
                        op=ALU.mult)
                    nc.vector.tensor_copy(msg[:, h0:h1, IN:IN + nhead],
                                          wv[:, h0:h1, :])
                    for j in range(h0, h1):
                        nc.tensor.matmul(bp, lhsT=S_all[:, j, :],
                                         rhs=msg[:, j, :],
                                         start=(j == 0), stop=(j == CPB - 1))
                # ---- epilogue
                rec = po.tile([P, nhead], f32, tag="rec")
                nc.vector.reciprocal(rec[:], bp[:, IN:IN + nhead])
                ti = po.tile([P, IN], f32, tag="ti")
                nc.vector.tensor_tensor(
                    ti[:].rearrange("p (h f) -> p h f", h=nhead),
                    bp[:, 0:IN].rearrange("p (h f) -> p h f", h=nhead),
                    rec[:].unsqueeze(2).to_broadcast(
                        [P, nhead, IN // nhead]),
                    op=ALU.mult)
                nc.vector.tensor_tensor(ti[:], ti[:], b_sb[:], op=ALU.add)
                tif = po.tile([P, IN], f16, tag="tif")
                if layer == 1:
                    # leaky_relu 0.01 then cast
                    tl = po.tile([P, IN], f32, tag="tl")
                    nc.vector.tensor_tensor(
                        tl[:], ti[:], c001[:].to_broadcast([P, IN]),
                        op=ALU.mult)
                    nc.vector.tensor_tensor(tif[:], tl[:], ti[:], op=ALU.max)
                else:
                    nc.scalar.activation(tif[:], ti[:], AF.Copy)
                hT = po.tile([P, 2, P], f16, tag="hT")
                nc.sync.dma_start_transpose(hT[:], tif[:])

                if layer == 1:
                    hp = ph.tile([P, AUG2], f32, tag="hp")
                    for k in range(2):
                        nc.tensor.matmul(hp[:], lhsT=hT[:, k, :],
                                         rhs=w2_sb[:, k, :],
                                         start=(k == 0), stop=(k == 1))
                    row2 = po.tile([P, IN + 2], f16, tag="row2")
                    nc.scalar.activation(row2[:, 0:IN], hp[:, 0:IN], AF.Copy)
                    r2f = row2[:].bitcast(f32)
                    nc.vector.tensor_copy(r2f[:, IN // 2:IN // 2 + 1],
                                          hp[:, IN:IN + 1])
                    nc.vector.tensor_copy(adL2[:, blk, :],
                                          hp[:, IN + 1:IN + 2])
                    nc.sync.dma_start(cc2_in[blk * P:(blk + 1) * P, 0:IN + 2],
                                      row2[:])
                else:
                    # fused cosine head: out = h2 @ [g | gm]
                    op_ = ph.tile([P, KH * MD], f32, tag="op")
                    for k in range(2):
                        nc.tensor.matmul(op_[:, 0:512], lhsT=hT[:, k, :],
                                         rhs=g_sb[:, k, 0:512],
                                         start=(k == 0), stop=(k == 1))
                    for k in range(2):
                        nc.tensor.matmul(op_[:, 512:KH * MD],
                                         lhsT=hT[:, k, :],
                                         rhs=g_sb[:, k, 512:KH * MD],
                                         start=(k == 0), stop=(k == 1))
                    num_ps = pn.tile([P, KH], f32, tag="nm")
                    num_ps = num_ps[:]
                    for k in range(2):
                        nc.tensor.matmul(num_ps, lhsT=hT[:, k, :],
                                         rhs=g_sb[:, k, KH * MD:GAUG],
                                         start=(k == 0), stop=(k == 1))
                    for k in range(KH):
                        sqs = po.tile([P, MD], f16, tag="sqs")
                        nc.scalar.activation(
                            sqs[:], op_[:, k * MD:(k + 1) * MD], AF.Square,
                            accum_out=nrm2_st[:, blk, k:k + 1])
                    nc.vector.tensor_copy(num_st[:, blk, :], num_ps)

            es.close()

        phase_a(1)
        edge_phase(1)
        phase_a(2)
        nc.gpsimd.collective_compute(
            "AllGather", ALU.bypass, replica_groups=[list(range(W))],
            ins=[cc2_in[:]], outs=[cc2_out[:]])
        edge_phase(2)

        # ---------------- final cosine ----------------------------------
        with tc.tile_pool(name="fin", bufs=1) as fin:
            nrm = fin.tile([P, NBLK, KH], f32)
            nc.scalar.activation(nrm[:], nrm2_st[:], AF.Sqrt)
            nc.vector.tensor_tensor(
                nrm[:], nrm[:],
                cmu_sb[:].unsqueeze(1).to_broadcast([P, NBLK, KH]),
                op=ALU.mult)
            nc.vector.tensor_scalar(out=nrm[:], in0=nrm[:], scalar1=1e-8,
                                    scalar2=None, op0=ALU.max)
            rcp = fin.tile([P, NBLK, KH], f32)
            nc.vector.reciprocal(rcp[:], nrm[:])
            res = fin.tile([P, NBLK, KH], f32)
            nc.vector.tensor_tensor(res[:], num_st[:], rcp[:], op=ALU.mult)
            nc.sync.dma_start(
                outD[:].rearrange("(b p) k -> p b k", p=P), res[:])

    nc.compile()
    return nc


# ===================== host-side preparation ============================

def _wrap16(flat):
    """flat idx [n] -> wrapped int16 [128, n//16] (8 Q7-core replicas)."""
    n = len(flat)
    out = np.zeros((P, n // 16), np.int16)
    cols = np.arange(n) // 16
    rows = np.arange(n) % 16
    for r in range(8):
        out[r * 16 + rows, cols] = flat
    return out


def prep_host(x, edge_index, W1, a_src1, a_dst1, b1, W2, a_src2, a_dst2, b2,
              g, mu):
    x = np.asarray(x, np.float32)
    N = x.shape[0]
    src = np.asarray(edge_index[0], np.int64)
    dst = np.asarray(edge_index[1], np.int64)
    E = len(src)

    od = np.bincount(src, minlength=N)
    idg = np.bincount(dst, minlength=N)

    # node -> core: top-5/8 by out-degree to cores 0-4 (lo table half),
    # in-degree snake within each group for compute balance.
    PERCORE = N // W
    order_od = np.argsort(-od, kind="stable")
    groupA = order_od[:5 * PERCORE]
    groupB = order_od[5 * PERCORE:]
    core_of = np.empty(N, np.int32)
    for base, grp in ((0, groupA), (5, groupB)):
        ncg = len(grp) // PERCORE
        gs = grp[np.argsort(-idg[grp], kind="stable")]
        pat = np.concatenate([np.arange(ncg), np.arange(ncg)[::-1]])
        asn = np.tile(pat, (len(gs) + 2 * ncg - 1) // (2 * ncg))[:len(gs)]
        core_of[gs] = base + asn

    # per-edge lo flag: src in cores 0-4
    src_lo = core_of[src] <= 4

    # per-node lo/hi in-degree
    idl = np.bincount(dst[src_lo], minlength=N)
    idh = idg - idl

    # per-core block packing under (lo, hi, count) caps
    loS = max(idl[core_of == c].sum() for c in range(W))
    hiS = max(idh[core_of == c].sum() for c in range(W))
    cands = []
    for tot in range(16, 24):
        for cl in range(1, tot):
            ch = tot - cl
            if loS <= 0.975 * NBLK * cl * P and hiS <= 0.975 * NBLK * ch * P:
                cands.append((cl, ch))
    ci = 0
    while True:
        CPL, CPH = cands[ci]
        capL, capH = CPL * P, CPH * P
        blk_of = np.full(N, -1, np.int32)
        slot_of = np.full(N, -1, np.int32)
        ok = True
        for c in range(W):
            nodes = np.where(core_of == c)[0]
            nodes = nodes[np.argsort(-(idl[nodes] + idh[nodes]),
                                     kind="stable")]
            bl = np.zeros(NBLK, np.int64)
            bh = np.zeros(NBLK, np.int64)
            bn = np.zeros(NBLK, np.int64)
            for n in nodes:
                load = np.maximum((bl + idl[n]) / capL, (bh + idh[n]) / capH)
                load[bn >= P] = 10.0
                load[(bl + idl[n]) > capL] = 10.0
                load[(bh + idh[n]) > capH] = 10.0
                b = int(np.argmin(load))
                if load[b] >= 10.0:
                    ok = False
                    break
                blk_of[n] = b
                slot_of[n] = bn[b]
                bl[b] += idl[n]
                bh[b] += idh[n]
                bn[b] += 1
            if not ok:
                break
        if ok:
            break
        ci += 1

    cfg = CFG(N=N, CPL=CPL, CPH=CPH, GAUG=KH * MD + KH)
    CPG, CPB = cfg.CPG, cfg.CPB

    row_of = core_of.astype(np.int64) * CAP + blk_of * P + slot_of

    # group edges by (core, block), lo first then hi, sorted by src row
    gkey = core_of[dst].astype(np.int64) * NBLK + blk_of[dst]
    skey = gkey * 2 + (~src_lo)
    order = np.argsort(skey * NROWS + row_of[src], kind="stable")
    esrc_r = row_of[src][order]
    edst_l = slot_of[dst][order].astype(np.int64)
    eslo = src_lo[order]
    ekey = gkey[order]
    starts = np.zeros(W * NBLK + 1, np.int64)
    cnts = np.bincount(ekey, minlength=W * NBLK)
    starts[1:] = np.cumsum(cnts)
    lo_cnt = np.bincount(ekey[eslo], minlength=W * NBLK)

    import ml_dtypes
    isrc_all = []
    dstf_all = []
    s2h_all = []
    for c in range(W):
        isrc = np.full((P, NBLK * CPG * 8), -1, np.int16)
        dstf = np.full((P, NBLK * CPB), -1.0, np.float16)
        dint = np.full((P, NBLK * CPB), -1, np.int64)
        for b in range(NBLK):
            gid = c * NBLK + b
            s0, s1 = starts[gid], starts[gid + 1]
            nlo = int(lo_cnt[gid])
            rows_ = esrc_r[s0:s1]
            dl = edst_l[s0:s1]
            fl = np.zeros(CPG * P, np.int64)   # pad: fetch row 0 (valid)
            fd = np.full(CPG * P, -1, np.int64)
            fl[:nlo] = rows_[:nlo]
            fd[:nlo] = dl[:nlo]
            nh = (s1 - s0) - nlo
            fl[CPL * P:CPL * P + nh] = rows_[nlo:] - LOSPLIT
            fd[CPL * P:CPL * P + nh] = dl[nlo:]
            isrc[:, b * CPG * 8:(b + 1) * CPG * 8] = _wrap16(fl)
            dcol = fd.reshape(CPG, P).T.astype(np.float16)
            dstf[:, b * CPB:b * CPB + CPG] = dcol
            dstf[:, b * CPB + CPG] = np.arange(P, dtype=np.float16)
            dint[:, b * CPB:b * CPB + CPG] = fd.reshape(CPG, P).T
            dint[:, b * CPB + CPG] = np.arange(P)
        # transposed one-hot: S2h[d, (b j e)] = 1 iff dst(e-slot, b, j) == d
        s2 = np.zeros((P, NBLK * CPB * P), ml_dtypes.bfloat16)
        ee, bj = np.nonzero(dint >= 0)
        s2[dint[ee, bj], bj * P + ee] = 1
        isrc_all.append(isrc)
        dstf_all.append(dstf)
        s2h_all.append(s2)

    # weights
    W1 = np.asarray(W1, np.float32)
    W2 = np.asarray(W2, np.float32)
    W1r = W1.reshape(H1, MD, IN)
    Ps1 = np.einsum("hdi,hd->ih", W1r, np.asarray(a_src1, np.float32))
    Pd1 = np.einsum("hdi,hd->ih", W1r, np.asarray(a_dst1, np.float32))
    W1aug = np.concatenate([W1.T, Ps1, Pd1], axis=1)
    Ps2 = W2.T @ np.asarray(a_src2, np.float32)[0][:, None]
    Pd2 = W2.T @ np.asarray(a_dst2, np.float32)[0][:, None]
    W2aug = np.concatenate([W2.T, Ps2, Pd2], axis=1)
    AUG1, AUG2 = IN + 4, IN + 2
    w1s = W1aug.reshape(2, P, AUG1).transpose(1, 0, 2).astype(np.float16)
    w2s = W2aug.reshape(2, P, AUG2).transpose(1, 0, 2).astype(np.float16)

    gm = np.asarray(g, np.float32)
    mu = np.asarray(mu, np.float32)
    gmu = np.einsum("fkm,km->fk", gm.reshape(IN, KH, MD), mu)  # [256, 8]
    g_aug = np.concatenate([gm, gmu], axis=1)                  # [256, 1032]
    gsd = g_aug.reshape(2, P, cfg.GAUG).transpose(1, 0, 2).astype(np.float16)
    cmu = np.broadcast_to(np.linalg.norm(mu, axis=1), (P, KH)).astype(
        np.float32).copy()
    b1b = np.broadcast_to(np.asarray(b1, np.float32), (P, HID)).copy()
    b2b = np.broadcast_to(np.asarray(b2, np.float32), (P, OUT)).copy()
    iota = np.broadcast_to(np.arange(P, dtype=np.float16), (P, P)).copy()

    shared = dict(w1s=w1s, w2s=w2s, gs=gsd, cmu=cmu, b1b=b1b, b2b=b2b,
                  iota=iota)
    in_maps = []
    for c in range(W):
        nodes = np.where(core_of == c)[0]
        xp = np.zeros((CAP, IN), np.float32)
        xp[blk_of[nodes] * P + slot_of[nodes]] = x[nodes]
        xTc = xp.reshape(NBLK, P, 2, P).transpose(3, 0, 2, 1).astype(
            np.float16)
        m = dict(shared)
        m.update(xTi=xTc, isrc=isrc_all[c], dstf=dstf_all[c],
                 s2h=s2h_all[c])
        in_maps.append(m)
    return cfg, in_maps, row_of


def assemble(outs, row_of, N):
    full = np.zeros((N, KH), np.float32)
    core = row_of // CAP
    rrow = row_of % CAP
    for c in range(W):
        sel = core == c
        full[sel] = outs[c]["outD"][rrow[sel]]
    return full


_CACHE = {}


def kernel(**inputs):
    cfg, in_maps, row_of = prep_host(**inputs)
    key = (cfg.N, cfg.CPL, cfg.CPH)
    if key not in _CACHE:
        _CACHE[key] = build_program(cfg)
    nc = _CACHE[key]
    from concourse.bass_utils import run_bass_kernel_spmd
    res = run_bass_kernel_spmd(nc, in_maps, core_ids=list(range(W)))
    return assemble(res.results, row_of, cfg.N)


# revision 19
# speedup vs baseline: 1.1859x; 1.0759x over previous
"""Trainium2 Bass kernel for nn_NodeInference (2-layer GAT + cosine head).

v4 design (SPMD, 8 cores, dst-node sharding, unified shard-order tables):
  Node n lives at shard row r(n) = core*6272 + block*128 + slot; the SAME row
  serves both GAT layers' tables, so one index set drives both edge phases.
  Table rows are 384 f16 (768B): [h 256xf16 | a_src f32 x nhead | pad].

  P1   dense-1 for OWN shard only -> cc1_in rows; a_dst(local) -> SBUF.
  AG1  AllGather cc1_in -> cc1_out (the layer-1 gather table).
  E1   per dst block (49): 17 gathered chunks of 128 edges (11 lo + 6 hi,
       Q7 dma_gather, 768B rows, int16 idx split at row 31360) + 1 contiguous
       self chunk from cc1_in. Batched DVE ops build the one-hot S, messages
       w*h, and edge weights w = exp(lrelu(a_s + a_d, 0.2)); a_d per edge via
       tiny S2^T @ adb matmuls where S2 = dma_start_transpose(S).
       PSUM accumulates S^T @ [msg | w] -> out1 -> leakyrelu(.,0.01) ->
       dense-2 -> cc2_in rows; a_dst2(local) -> SBUF.
  AG2  AllGather cc2_in -> cc2_out.
  E2   same for layer 2 (1 head); epilogue fuses the cosine head per block:
       out = h2 @ [g | g@mu-diag], nrm2 via square+reduce, staged to SBUF.
  F    one final pass: cos = num / max(nrm*|mu|, 1e-8) -> outD [6272, 8].
Host: balanced node->core (in-degree snake within out-degree groups) and
node->block packing (greedy FFD under lo/hi chunk caps).
"""

import sys
from dataclasses import dataclass
from contextlib import ExitStack

if "/opt/trn_rl_repo" not in sys.path:
    sys.path.insert(0, "/opt/trn_rl_repo")

import numpy as np

import concourse.bacc as bacc
import concourse.bass as bass
import concourse.mybir as mybir
import concourse.tile as tile

P = 128
IN = 256
H1 = 2            # layer-1 heads
HID = 256         # layer-1 out dim (2*128 concat)
OUT = 256         # layer-2 out dim
KH, MD = 8, 128   # cosine head
ROWW = 384        # f16 cols per table row (768B)
W = 8             # world size
NBLK = 49
CAP = NBLK * P    # 6272 rows per shard
NROWS = W * CAP   # 50176
LOSPLIT = 31360   # lo/hi table split (5 shards; both halves < 32768 rows)
AF = mybir.ActivationFunctionType
ALU = mybir.AluOpType
DT = mybir.dt


@dataclass
class CFG:
    N: int
    CPL: int   # lo chunks per block
    CPH: int   # hi chunks per block
    GAUG: int  # g_aug cols (1024 + 8)

    @property
    def CPG(self):  # gathered chunks
        return self.CPL + self.CPH

    @property
    def CPB(self):  # total chunks incl self
        return self.CPG + 1


def build_program(cfg: CFG):
    nc = bacc.Bacc("TRN2", target_bir_lowering=False, debug=False,
                   num_swdge_queues=4, dynamic_dma_scratch_size=65536)
    CPL, CPH, CPG, CPB = cfg.CPL, cfg.CPH, cfg.CPG, cfg.CPB
    AUG1, AUG2 = IN + 4, IN + 2
    GAUG = cfg.GAUG
    f16, f32, i16 = DT.float16, DT.float32, DT.int16
    bf16 = DT.bfloat16

    with tile.TileContext(nc) as tc, ExitStack() as stack:
        dram = stack.enter_context(
            tc.tile_pool(name="dram", bufs=1, space="DRAM"))

        def din(name, shape, dtype):
            return dram.tile(shape, dtype, kind="ExternalInput", name=name,
                             uniquify=False)

        xTi = din("xTi", [P, NBLK, 2, P], f16)
        w1s = din("w1s", [P, 2, AUG1], f16)
        w2s = din("w2s", [P, 2, AUG2], f16)
        gsd = din("gs", [P, 2, GAUG], f16)
        cmud = din("cmu", [P, KH], f32)       # |mu_k| host-broadcast
        b1d = din("b1b", [P, HID], f32)
        b2d = din("b2b", [P, OUT], f32)
        iotd = din("iota", [P, P], f16)       # iota[p, f] = f
        isrd = din("isrc", [P, NBLK * CPG * 8], i16)
        dstd = din("dstf", [P, NBLK * CPB], f16)
        s2hd = din("s2h", [P, NBLK * CPB * P], DT.bfloat16)
        shd = din("sh", [P, NBLK * CPB * P], DT.bfloat16)
        outD = dram.tile([CAP, KH], f32, kind="ExternalOutput", name="outD",
                         uniquify=False)

        cc1_in = dram.tile([CAP, ROWW], f16, name="cc1_in")
        cc2_in = dram.tile([CAP, ROWW], f16, name="cc2_in")
        cc1_out = dram.tile([NROWS, ROWW], f16, name="cc1_out",
                            addr_space="Shared")
        cc2_out = dram.tile([NROWS, ROWW], f16, name="cc2_out",
                            addr_space="Shared")

        consts = stack.enter_context(tc.tile_pool(name="consts", bufs=1))
        w1_sb = consts.tile([P, 2, AUG1], f16)
        w2_sb = consts.tile([P, 2, AUG2], f16)
        g_sb = consts.tile([P, 2, GAUG], f16)
        cmu_sb = consts.tile([P, KH], f32)
        b1_sb = consts.tile([P, HID], f32)
        b2_sb = consts.tile([P, OUT], f32)
        iota_sb = consts.tile([P, P], f16)
        adL1 = consts.tile([P, NBLK, 2], bf16)   # a_dst of local nodes, L1
        adL2 = consts.tile([P, NBLK, 1], bf16)   # a_dst of local nodes, L2
        nrm2_st = consts.tile([P, NBLK, KH], f32)
        num_st = consts.tile([P, NBLK, KH], f32)
        c02 = consts.tile([P, 1], f32)
        c001 = consts.tile([P, 1], f32)
        nc.vector.memset(c02[:], 0.2)
        nc.vector.memset(c001[:], 0.01)

        for d, s in [(w1_sb, w1s), (w2_sb, w2s), (g_sb, gsd),
                     (cmu_sb, cmud), (b1_sb, b1d), (b2_sb, b2d),
                     (iota_sb, iotd)]:
            nc.sync.dma_start(d[:], s[:])

        # ---------------- P1: dense layer 1 (own shard only) ------------
        with tc.tile_pool(name="p1x", bufs=3) as p1x, \
             tc.tile_pool(name="p1ps", bufs=2, space="PSUM") as p1ps, \
             tc.tile_pool(name="p1row", bufs=3) as p1row:
            for t in range(NBLK):
                xt = p1x.tile([P, 2, P], f16, tag="xt")
                nc.sync.dma_start(xt[:], xTi[:, t, :, :])
                ps = p1ps.tile([P, AUG1], f32, tag="ps")
                for k in range(2):
                    nc.tensor.matmul(ps[:], lhsT=xt[:, k, :],
                                     rhs=w1_sb[:, k, :],
                                     start=(k == 0), stop=(k == 1))
                row = p1row.tile([P, IN + 4], f16, tag="row")
                nc.scalar.activation(row[:, 0:IN], ps[:, 0:IN], AF.Copy)
                rf = row[:].bitcast(f32)
                nc.vector.tensor_copy(rf[:, IN // 2:IN // 2 + 2],
                                      ps[:, IN:IN + 2])
                nc.vector.tensor_copy(adL1[:, t, :], ps[:, IN + 2:IN + 4])
                nc.sync.dma_start(cc1_in[t * P:(t + 1) * P, 0:IN + 4],
                                  row[:])

        nc.gpsimd.collective_compute(
            "AllGather", ALU.bypass, replica_groups=[list(range(W))],
            ins=[cc1_in[:]], outs=[cc1_out[:]])

        gq = [0]  # global Pool-DMA emission counter: sem lane = gq%8,
        # so queue gq%4 keeps each DMASW sem pinned to one SWDGE queue

        # ---- per-edge a_dst staging (filled by pre-pass in AG windows)
        ad_st1 = consts.tile([P, NBLK, CPB, 2], f32)
        ad_st2 = consts.tile([P, NBLK, CPB, 1], f32)

        # shared SBUF pools for both layers (avoids an SBUF reallocation
        # barrier at phase boundaries that would stall the AG windows)
        pi = stack.enter_context(tc.tile_pool(name="idx", bufs=3))
        pg = stack.enter_context(tc.tile_pool(name="gath", bufs=4))
        pe_ = stack.enter_context(tc.tile_pool(name="ew", bufs=3))
        pm = stack.enter_context(tc.tile_pool(name="msg", bufs=3))
        po = stack.enter_context(tc.tile_pool(name="epi", bufs=2))
        pA = stack.enter_context(tc.tile_pool(name="pA", bufs=3))
        # init gather buffers once: stale data must stay finite
        for ig in range(4):
            g0 = pg.tile([P, CPB, ROWW], f16, tag="gt")
            nc.vector.memset(g0[:], 0.0)

        # ---------------- pre-pass: S2^T @ adb -> staged a_dst ----------
        def phase_a(layer):
            adL, adst, nhead = ((adL1, ad_st1, 2) if layer == 1
                                else (adL2, ad_st2, 1))
            with tc.tile_pool(name=f"pAp{layer}", bufs=2,
                              space="PSUM") as pAp:
                for blk in range(NBLK):
                    S2 = pA.tile([P, CPB, P], bf16, tag="S2")
                    nc.sync.dma_start(
                        S2[:], s2hd[:, blk * CPB * P:(blk + 1) * CPB * P]
                        .rearrange("p (a b) -> p a b", a=CPB))
                    ad_ps = pAp.tile([P, CPB * nhead], f32, tag="adp")
                    for j in range(CPB):
                        nc.tensor.matmul(
                            ad_ps[:, j * nhead:(j + 1) * nhead],
                            lhsT=S2[:, j, :], rhs=adL[:, blk, :],
                            start=True, stop=True)
                    nc.vector.tensor_copy(
                        adst[:, blk].rearrange("p a b -> p (a b)"), ad_ps[:])

        # ---------------- main pass ------------------------------------
        def edge_phase(layer):
            if layer == 1:
                table, self_tab = cc1_out, cc1_in
                nhead, MSGC, adst, b_sb = 2, HID + 2, ad_st1, b1_sb
            else:
                table, self_tab = cc2_out, cc2_in
                nhead, MSGC, adst, b_sb = 1, OUT + 1, ad_st2, b2_sb
            tab_lo = table[0:LOSPLIT, :]
            tab_hi = table[LOSPLIT:NROWS, :]

            es = ExitStack()
            pp = es.enter_context(
                tc.tile_pool(name=f"bps{layer}", bufs=2, space="PSUM"))
            ph = es.enter_context(
                tc.tile_pool(name=f"hps{layer}", bufs=2, space="PSUM"))
            pn = es.enter_context(
                tc.tile_pool(name=f"nps{layer}", bufs=2, space="PSUM"))

            # S one-hots built one block ahead of consumption
            S_t = {}

            def prep_S(b):
                S = pm.tile([P, CPB, P], bf16, tag="S")
                nc.sync.dma_start(
                    S[:], shd[:, b * CPB * P:(b + 1) * CPB * P]
                    .rearrange("p (a b) -> p a b", a=CPB))
                S_t[b] = S

            prep_S(0)
            for blk in range(NBLK):
                isrc = pi.tile([P, CPG * 8], i16, tag="isrc")
                nc.sync.dma_start(isrc[:],
                                  isrd[:, blk * CPG * 8:(blk + 1) * CPG * 8])

                gt = pg.tile([P, CPB, ROWW], f16, tag="gt")
                for c0 in range(0, CPL, 8):
                    c1 = min(c0 + 8, CPL)
                    nc.gpsimd.dma_gather(
                        gt[:, c0:c1, :], tab_lo,
                        isrc[:, c0 * 8:c1 * 8],
                        (c1 - c0) * P, (c1 - c0) * P, ROWW,
                        queue_num=gq[0] % 4)
                    gq[0] += 1
                for c0 in range(CPL, CPG, 8):
                    c1 = min(c0 + 8, CPG)
                    nc.gpsimd.dma_gather(
                        gt[:, c0:c1, :], tab_hi,
                        isrc[:, c0 * 8:c1 * 8],
                        (c1 - c0) * P, (c1 - c0) * P, ROWW,
                        queue_num=gq[0] % 4)
                    gq[0] += 1
                # self chunk: contiguous rows of own shard
                nc.sync.dma_start(gt[:, CPG, 0:IN + 2 * nhead],
                                  self_tab[blk * P:(blk + 1) * P,
                                           0:IN + 2 * nhead])

                if blk + 1 < NBLK:
                    prep_S(blk + 1)
                S_all = S_t.pop(blk)

                # ew = exp(lrelu(a_s + a_d, 0.2))
                gtf = gt[:].bitcast(f32)
                as_v = gtf[:, :, IN // 2:IN // 2 + nhead]
                ew_t = pe_.tile([P, CPB, 2], f32, tag="ew")
                ew = ew_t[:, :, 0:nhead]
                nc.vector.tensor_tensor(ew, as_v, adst[:, blk],
                                        op=ALU.add)
                ew2_t = pe_.tile([P, CPB, 2], f32, tag="ew2")
                ew2 = ew2_t[:, :, 0:nhead]
                nc.vector.tensor_tensor(
                    ew2, ew,
                    c02[:].unsqueeze(2).to_broadcast([P, CPB, nhead]),
                    op=ALU.mult)
                nc.vector.tensor_tensor(ew, ew2, ew, op=ALU.max)
                wv_t = pe_.tile([P, CPB, 2], bf16, tag="wv")
                wv = wv_t[:, :, 0:nhead]
                nc.scalar.activation(wv, ew, AF.Exp)

                # msg = [w*h | w], two halves so bp matmuls start early
                msg_t = pm.tile([P, CPB, HID + 2], bf16, tag="msg")
                msg = msg_t[:, :, 0:MSGC]
                bp_t = pp.tile([P, HID + 2], f32, tag="bp")
                bp = bp_t[:, 0:MSGC]
                HSPL = CPB // 2
                for h0, h1 in ((0, HSPL), (HSPL, CPB)):
                    nc.vector.tensor_tensor(
                        msg[:, h0:h1, 0:IN].rearrange(
                            "p a (h f) -> p a h f", h=nhead),
                        gt[:, h0:h1, 0:IN].rearrange(
                            "p a (h f) -> p a h f", h=nhead),
                        wv[:, h0:h1, :].unsqueeze(3).to_broadcast(
                            [P, h1 - h0, nhead, IN // nhead]),
                        op=ALU.mult)
                    nc.vector.tensor_copy(msg[:, h0:h1, IN:IN + nhead],
                                          wv[:, h0:h1, :])
                    for j in range(h0, h1):
                        nc.tensor.matmul(bp, lhsT=S_all[:, j, :],
                                         rhs=msg[:, j, :],
                                         start=(j == 0), stop=(j == CPB - 1))
                # ---- epilogue
                rec = po.tile([P, nhead], f32, tag="rec")
                nc.vector.reciprocal(rec[:], bp[:, IN:IN + nhead])
                ti = po.tile([P, IN], f32, tag="ti")
                nc.vector.tensor_tensor(
                    ti[:].rearrange("p (h f) -> p h f", h=nhead),
                    bp[:, 0:IN].rearrange("p (h f) -> p h f", h=nhead),
                    rec[:].unsqueeze(2).to_broadcast(
                        [P, nhead, IN // nhead]),
                    op=ALU.mult)
                nc.vector.tensor_tensor(ti[:], ti[:], b_sb[:], op=ALU.add)
                tif = po.tile([P, IN], f16, tag="tif")
                if layer == 1:
                    # leaky_relu 0.01 then cast
                    tl = po.tile([P, IN], f32, tag="tl")
                    nc.vector.tensor_tensor(
                        tl[:], ti[:], c001[:].to_broadcast([P, IN]),
                        op=ALU.mult)
                    nc.vector.tensor_tensor(tif[:], tl[:], ti[:], op=ALU.max)
                else:
                    nc.scalar.activation(tif[:], ti[:], AF.Copy)
                hT = po.tile([P, 2, P], f16, tag="hT")
                nc.sync.dma_start_transpose(hT[:], tif[:])

                if layer == 1:
                    hp = ph.tile([P, AUG2], f32, tag="hp")
                    for k in range(2):
                        nc.tensor.matmul(hp[:], lhsT=hT[:, k, :],
                                         rhs=w2_sb[:, k, :],
                                         start=(k == 0), stop=(k == 1))
                    row2 = po.tile([P, IN + 2], f16, tag="row2")
                    nc.scalar.activation(row2[:, 0:IN], hp[:, 0:IN], AF.Copy)
                    r2f = row2[:].bitcast(f32)
                    nc.vector.tensor_copy(r2f[:, IN // 2:IN // 2 + 1],
                                          hp[:, IN:IN + 1])
                    nc.scalar.activation(adL2[:, blk, :],
                                         hp[:, IN + 1:IN + 2], AF.Copy)
                    nc.sync.dma_start(cc2_in[blk * P:(blk + 1) * P, 0:IN + 2],
                                      row2[:])
                else:
                    # fused cosine head: out = h2 @ [g | gm]
                    op_ = ph.tile([P, KH * MD], f32, tag="op")
                    for k in range(2):
                        nc.tensor.matmul(op_[:, 0:512], lhsT=hT[:, k, :],
                                         rhs=g_sb[:, k, 0:512],
                                         start=(k == 0), stop=(k == 1))
                    for k in range(2):
                        nc.tensor.matmul(op_[:, 512:KH * MD],
                                         lhsT=hT[:, k, :],
                                         rhs=g_sb[:, k, 512:KH * MD],
                                         start=(k == 0), stop=(k == 1))
                    num_ps = pn.tile([P, KH], f32, tag="nm")
                    num_ps = num_ps[:]
                    for k in range(2):
                        nc.tensor.matmul(num_ps, lhsT=hT[:, k, :],
                                         rhs=g_sb[:, k, KH * MD:GAUG],
                                         start=(k == 0), stop=(k == 1))
                    for k in range(KH):
                        sqs = po.tile([P, MD], f16, tag="sqs")
                        nc.scalar.activation(
                            sqs[:], op_[:, k * MD:(k + 1) * MD], AF.Square,
                            accum_out=nrm2_st[:, blk, k:k + 1])
                    nc.scalar.activation(num_st[:, blk, :], num_ps, AF.Copy)

            es.close()

        phase_a(1)
        edge_phase(1)
        phase_a(2)
        nc.gpsimd.collective_compute(
            "AllGather", ALU.bypass, replica_groups=[list(range(W))],
            ins=[cc2_in[:]], outs=[cc2_out[:]])
        edge_phase(2)

        # ---------------- final cosine ----------------------------------
        with tc.tile_pool(name="fin", bufs=1) as fin:
            nrm = fin.tile([P, NBLK, KH], f32)
            nc.scalar.activation(nrm[:], nrm2_st[:], AF.Sqrt)
            nc.vector.tensor_tensor(
                nrm[:], nrm[:],
                cmu_sb[:].unsqueeze(1).to_broadcast([P, NBLK, KH]),
                op=ALU.mult)
            nc.vector.tensor_scalar(out=nrm[:], in0=nrm[:], scalar1=1e-8,
                                    scalar2=None, op0=ALU.max)
            rcp = fin.tile([P, NBLK, KH], f32)
            nc.vector.reciprocal(rcp[:], nrm[:])
            res = fin.tile([P, NBLK, KH], f32)
            nc.vector.tensor_tensor(res[:], num_st[:], rcp[:], op=ALU.mult)
            nc.sync.dma_start(
                outD[:].rearrange("(b p) k -> p b k", p=P), res[:])

    nc.compile()
    return nc


# ===================== host-side preparation ============================

def _wrap16(flat):
    """flat idx [n] -> wrapped int16 [128, n//16] (8 Q7-core replicas)."""
    n = len(flat)
    out = np.zeros((P, n // 16), np.int16)
    cols = np.arange(n) // 16
    rows = np.arange(n) % 16
    for r in range(8):
        out[r * 16 + rows, cols] = flat
    return out


def prep_host(x, edge_index, W1, a_src1, a_dst1, b1, W2, a_src2, a_dst2, b2,
              g, mu):
    x = np.asarray(x, np.float32)
    N = x.shape[0]
    src = np.asarray(edge_index[0], np.int64)
    dst = np.asarray(edge_index[1], np.int64)
    E = len(src)

    od = np.bincount(src, minlength=N)
    idg = np.bincount(dst, minlength=N)

    # node -> core: top-5/8 by out-degree to cores 0-4 (lo table half),
    # in-degree snake within each group for compute balance.
    PERCORE = N // W
    order_od = np.argsort(-od, kind="stable")
    groupA = order_od[:5 * PERCORE]
    groupB = order_od[5 * PERCORE:]
    core_of = np.empty(N, np.int32)
    for base, grp in ((0, groupA), (5, groupB)):
        ncg = len(grp) // PERCORE
        gs = grp[np.argsort(-idg[grp], kind="stable")]
        pat = np.concatenate([np.arange(ncg), np.arange(ncg)[::-1]])
        asn = np.tile(pat, (len(gs) + 2 * ncg - 1) // (2 * ncg))[:len(gs)]
        core_of[gs] = base + asn

    # per-edge lo flag: src in cores 0-4
    src_lo = core_of[src] <= 4

    # per-node lo/hi in-degree
    idl = np.bincount(dst[src_lo], minlength=N)
    idh = idg - idl

    # per-core block packing under (lo, hi, count) caps
    loS = max(idl[core_of == c].sum() for c in range(W))
    hiS = max(idh[core_of == c].sum() for c in range(W))
    cands = []
    for tot in range(16, 24):
        for cl in range(1, tot):
            ch = tot - cl
            if loS <= 0.975 * NBLK * cl * P and hiS <= 0.975 * NBLK * ch * P:
                cands.append((cl, ch))
    ci = 0
    while True:
        CPL, CPH = cands[ci]
        capL, capH = CPL * P, CPH * P
        blk_of = np.full(N, -1, np.int32)
        slot_of = np.full(N, -1, np.int32)
        ok = True
        for c in range(W):
            nodes = np.where(core_of == c)[0]
            nodes = nodes[np.argsort(-(idl[nodes] + idh[nodes]),
                                     kind="stable")]
            bl = np.zeros(NBLK, np.int64)
            bh = np.zeros(NBLK, np.int64)
            bn = np.zeros(NBLK, np.int64)
            for n in nodes:
                load = np.maximum((bl + idl[n]) / capL, (bh + idh[n]) / capH)
                load[bn >= P] = 10.0
                load[(bl + idl[n]) > capL] = 10.0
                load[(bh + idh[n]) > capH] = 10.0
                b = int(np.argmin(load))
                if load[b] >= 10.0:
                    ok = False
                    break
                blk_of[n] = b
                slot_of[n] = bn[b]
                bl[b] += idl[n]
                bh[b] += idh[n]
                bn[b] += 1
            if not ok:
                break
        if ok:
            break
        ci += 1

    cfg = CFG(N=N, CPL=CPL, CPH=CPH, GAUG=KH * MD + KH)
    CPG, CPB = cfg.CPG, cfg.CPB

    row_of = core_of.astype(np.int64) * CAP + blk_of * P + slot_of

    # group edges by (core, block), lo first then hi, sorted by src row
    gkey = core_of[dst].astype(np.int64) * NBLK + blk_of[dst]
    skey = gkey * 2 + (~src_lo)
    order = np.argsort(skey * NROWS + row_of[src], kind="stable")
    esrc_r = row_of[src][order]
    edst_l = slot_of[dst][order].astype(np.int64)
    eslo = src_lo[order]
    ekey = gkey[order]
    starts = np.zeros(W * NBLK + 1, np.int64)
    cnts = np.bincount(ekey, minlength=W * NBLK)
    starts[1:] = np.cumsum(cnts)
    lo_cnt = np.bincount(ekey[eslo], minlength=W * NBLK)

    import ml_dtypes
    isrc_all = []
    dstf_all = []
    s2h_all = []
    sh_all = []
    for c in range(W):
        isrc = np.full((P, NBLK * CPG * 8), -1, np.int16)
        dstf = np.full((P, NBLK * CPB), -1.0, np.float16)
        dint = np.full((P, NBLK * CPB), -1, np.int64)
        for b in range(NBLK):
            gid = c * NBLK + b
            s0, s1 = starts[gid], starts[gid + 1]
            nlo = int(lo_cnt[gid])
            rows_ = esrc_r[s0:s1]
            dl = edst_l[s0:s1]
            fl = np.zeros(CPG * P, np.int64)   # pad: fetch row 0 (valid)
            fd = np.full(CPG * P, -1, np.int64)
            fl[:nlo] = rows_[:nlo]
            fd[:nlo] = dl[:nlo]
            nh = (s1 - s0) - nlo
            fl[CPL * P:CPL * P + nh] = rows_[nlo:] - LOSPLIT
            fd[CPL * P:CPL * P + nh] = dl[nlo:]
            isrc[:, b * CPG * 8:(b + 1) * CPG * 8] = _wrap16(fl)
            dcol = fd.reshape(CPG, P).T.astype(np.float16)
            dstf[:, b * CPB:b * CPB + CPG] = dcol
            dstf[:, b * CPB + CPG] = np.arange(P, dtype=np.float16)
            dint[:, b * CPB:b * CPB + CPG] = fd.reshape(CPG, P).T
            dint[:, b * CPB + CPG] = np.arange(P)
        # transposed one-hot: S2h[d, (b j e)] = 1 iff dst(e-slot, b, j) == d
        s2 = np.zeros((P, NBLK * CPB * P), ml_dtypes.bfloat16)
        ee, bj = np.nonzero(dint >= 0)
        s2[dint[ee, bj], bj * P + ee] = 1
        sh = np.zeros((P, NBLK * CPB * P), ml_dtypes.bfloat16)
        sh[ee, bj * P + dint[ee, bj]] = 1
        isrc_all.append(isrc)
        dstf_all.append(dstf)
        s2h_all.append(s2)
        sh_all.append(sh)

    # weights
    W1 = np.asarray(W1, np.float32)
    W2 = np.asarray(W2, np.float32)
    W1r = W1.reshape(H1, MD, IN)
    Ps1 = np.einsum("hdi,hd->ih", W1r, np.asarray(a_src1, np.float32))
    Pd1 = np.einsum("hdi,hd->ih", W1r, np.asarray(a_dst1, np.float32))
    W1aug = np.concatenate([W1.T, Ps1, Pd1], axis=1)
    Ps2 = W2.T @ np.asarray(a_src2, np.float32)[0][:, None]
    Pd2 = W2.T @ np.asarray(a_dst2, np.float32)[0][:, None]
    W2aug = np.concatenate([W2.T, Ps2, Pd2], axis=1)
    AUG1, AUG2 = IN + 4, IN + 2
    w1s = W1aug.reshape(2, P, AUG1).transpose(1, 0, 2).astype(np.float16)
    w2s = W2aug.reshape(2, P, AUG2).transpose(1, 0, 2).astype(np.float16)

    gm = np.asarray(g, np.float32)
    mu = np.asarray(mu, np.float32)
    gmu = np.einsum("fkm,km->fk", gm.reshape(IN, KH, MD), mu)  # [256, 8]
    g_aug = np.concatenate([gm, gmu], axis=1)                  # [256, 1032]
    gsd = g_aug.reshape(2, P, cfg.GAUG).transpose(1, 0, 2).astype(np.float16)
    cmu = np.broadcast_to(np.linalg.norm(mu, axis=1), (P, KH)).astype(
        np.float32).copy()
    b1b = np.broadcast_to(np.asarray(b1, np.float32), (P, HID)).copy()
    b2b = np.broadcast_to(np.asarray(b2, np.float32), (P, OUT)).copy()
    iota = np.broadcast_to(np.arange(P, dtype=np.float16), (P, P)).copy()

    shared = dict(w1s=w1s, w2s=w2s, gs=gsd, cmu=cmu, b1b=b1b, b2b=b2b,
                  iota=iota)
    in_maps = []
    for c in range(W):
        nodes = np.where(core_of == c)[0]
        xp = np.zeros((CAP, IN), np.float32)
        xp[blk_of[nodes] * P + slot_of[nodes]] = x[nodes]
        xTc = xp.reshape(NBLK, P, 2, P).transpose(3, 0, 2, 1).astype(
            np.float16)
        m = dict(shared)
        m.update(xTi=xTc, isrc=isrc_all[c], dstf=dstf_all[c],
                 s2h=s2h_all[c], sh=sh_all[c])
        in_maps.append(m)
    return cfg, in_maps, row_of


def assemble(outs, row_of, N):
    full = np.zeros((N, KH), np.float32)
    core = row_of // CAP
    rrow = row_of % CAP
    for c in range(W):
        sel = core == c
        full[sel] = outs[c]["outD"][rrow[sel]]
    return full


_CACHE = {}


def kernel(**inputs):
    cfg, in_maps, row_of = prep_host(**inputs)
    key = (cfg.N, cfg.CPL, cfg.CPH)
    if key not in _CACHE:
        _CACHE[key] = build_program(cfg)
    nc = _CACHE[key]
    from concourse.bass_utils import run_bass_kernel_spmd
    res = run_bass_kernel_spmd(nc, in_maps, core_ids=list(range(W)))
    return assemble(res.results, row_of, cfg.N)


# revision 21
# speedup vs baseline: 1.2468x; 1.0513x over previous
"""Trainium2 Bass kernel for nn_NodeInference (2-layer GAT + cosine head).

v4 design (SPMD, 8 cores, dst-node sharding, unified shard-order tables):
  Node n lives at shard row r(n) = core*6272 + block*128 + slot; the SAME row
  serves both GAT layers' tables, so one index set drives both edge phases.
  Table rows are 384 f16 (768B): [h 256xf16 | a_src f32 x nhead | pad].

  P1   dense-1 for OWN shard only -> cc1_in rows; a_dst(local) -> SBUF.
  AG1  AllGather cc1_in -> cc1_out (the layer-1 gather table).
  E1   per dst block (49): 17 gathered chunks of 128 edges (11 lo + 6 hi,
       Q7 dma_gather, 768B rows, int16 idx split at row 31360) + 1 contiguous
       self chunk from cc1_in. Batched DVE ops build the one-hot S, messages
       w*h, and edge weights w = exp(lrelu(a_s + a_d, 0.2)); a_d per edge via
       tiny S2^T @ adb matmuls where S2 = dma_start_transpose(S).
       PSUM accumulates S^T @ [msg | w] -> out1 -> leakyrelu(.,0.01) ->
       dense-2 -> cc2_in rows; a_dst2(local) -> SBUF.
  AG2  AllGather cc2_in -> cc2_out.
  E2   same for layer 2 (1 head); epilogue fuses the cosine head per block:
       out = h2 @ [g | g@mu-diag], nrm2 via square+reduce, staged to SBUF.
  F    one final pass: cos = num / max(nrm*|mu|, 1e-8) -> outD [6272, 8].
Host: balanced node->core (in-degree snake within out-degree groups) and
node->block packing (greedy FFD under lo/hi chunk caps).
"""

import sys
from dataclasses import dataclass
from contextlib import ExitStack

if "/opt/trn_rl_repo" not in sys.path:
    sys.path.insert(0, "/opt/trn_rl_repo")

import numpy as np

import concourse.bacc as bacc
import concourse.bass as bass
import concourse.mybir as mybir
import concourse.tile as tile

P = 128
IN = 256
H1 = 2            # layer-1 heads
HID = 256         # layer-1 out dim (2*128 concat)
OUT = 256         # layer-2 out dim
KH, MD = 8, 128   # cosine head
ROWW = 384        # f16 cols per table row (768B)
W = 8             # world size
NBLK = 49
CAP = NBLK * P    # 6272 rows per shard
NROWS = W * CAP   # 50176
LOSPLIT = 31360   # lo/hi table split (5 shards; both halves < 32768 rows)
AF = mybir.ActivationFunctionType
ALU = mybir.AluOpType
DT = mybir.dt


@dataclass
class CFG:
    N: int
    CPL: int   # lo chunks per block
    CPH: int   # hi chunks per block
    GAUG: int  # g_aug cols (1024 + 8)

    @property
    def CPG(self):  # gathered chunks
        return self.CPL + self.CPH

    @property
    def CPB(self):  # total chunks incl self
        return self.CPG + 1


def build_program(cfg: CFG):
    nc = bacc.Bacc("TRN2", target_bir_lowering=False, debug=False,
                   num_swdge_queues=4, dynamic_dma_scratch_size=65536)
    CPL, CPH, CPG, CPB = cfg.CPL, cfg.CPH, cfg.CPG, cfg.CPB
    AUG1, AUG2 = IN + 4, IN + 2
    GAUG = cfg.GAUG
    f16, f32, i16 = DT.float16, DT.float32, DT.int16
    bf16 = DT.bfloat16
    f8 = DT.float8e4

    with tile.TileContext(nc) as tc, ExitStack() as stack:
        dram = stack.enter_context(
            tc.tile_pool(name="dram", bufs=1, space="DRAM"))

        def din(name, shape, dtype):
            return dram.tile(shape, dtype, kind="ExternalInput", name=name,
                             uniquify=False)

        xTi = din("xTi", [P, NBLK, 2, P], f16)
        w1s = din("w1s", [P, 2, AUG1], f16)
        w2s = din("w2s", [P, 2, AUG2], f16)
        gsd = din("gs", [P, 2, GAUG], f16)
        cmud = din("cmu", [P, KH], f32)       # |mu_k| host-broadcast
        b1d = din("b1b", [P, HID], f32)
        b2d = din("b2b", [P, OUT], f32)
        iotd = din("iota", [P, P], f16)       # iota[p, f] = f
        isrd = din("isrc", [P, NBLK * CPG * 8], i16)
        dstd = din("dstf", [P, NBLK * CPB], f16)
        s2hd = din("s2h", [P, NBLK * CPB * P], f8)
        shd = din("sh", [P, NBLK * CPB * P], f8)
        outD = dram.tile([CAP, KH], f32, kind="ExternalOutput", name="outD",
                         uniquify=False)

        cc1_in = dram.tile([CAP, ROWW], f16, name="cc1_in")
        cc2_in = dram.tile([CAP, ROWW], f16, name="cc2_in")
        cc1_out = dram.tile([NROWS, ROWW], f16, name="cc1_out",
                            addr_space="Shared")
        cc2_out = dram.tile([NROWS, ROWW], f16, name="cc2_out",
                            addr_space="Shared")

        consts = stack.enter_context(tc.tile_pool(name="consts", bufs=1))
        w1_sb = consts.tile([P, 2, AUG1], f16)
        w2_sb = consts.tile([P, 2, AUG2], f16)
        g_sb = consts.tile([P, 2, GAUG], f16)
        cmu_sb = consts.tile([P, KH], f32)
        b1_sb = consts.tile([P, HID], f32)
        b2_sb = consts.tile([P, OUT], f32)
        iota_sb = consts.tile([P, P], f16)
        adL1 = consts.tile([P, NBLK, 2], bf16)   # a_dst of local nodes, L1
        adL2 = consts.tile([P, NBLK, 1], bf16)   # a_dst of local nodes, L2
        nrm2_st = consts.tile([P, NBLK, KH], f32)
        num_st = consts.tile([P, NBLK, KH], f32)
        c02 = consts.tile([P, 1], f32)
        c001 = consts.tile([P, 1], f32)
        nc.vector.memset(c02[:], 0.2)
        nc.vector.memset(c001[:], 0.01)

        for d, s in [(w1_sb, w1s), (w2_sb, w2s), (g_sb, gsd),
                     (cmu_sb, cmud), (b1_sb, b1d), (b2_sb, b2d),
                     (iota_sb, iotd)]:
            nc.sync.dma_start(d[:], s[:])

        # ---------------- P1: dense layer 1 (own shard only) ------------
        with tc.tile_pool(name="p1x", bufs=3) as p1x, \
             tc.tile_pool(name="p1ps", bufs=2, space="PSUM") as p1ps, \
             tc.tile_pool(name="p1row", bufs=3) as p1row:
            for t in range(NBLK):
                xt = p1x.tile([P, 2, P], f16, tag="xt")
                nc.sync.dma_start(xt[:], xTi[:, t, :, :])
                ps = p1ps.tile([P, AUG1], f32, tag="ps")
                for k in range(2):
                    nc.tensor.matmul(ps[:], lhsT=xt[:, k, :],
                                     rhs=w1_sb[:, k, :],
                                     start=(k == 0), stop=(k == 1))
                row = p1row.tile([P, IN + 4], f16, tag="row")
                nc.scalar.activation(row[:, 0:IN], ps[:, 0:IN], AF.Copy)
                rf = row[:].bitcast(f32)
                nc.vector.tensor_copy(rf[:, IN // 2:IN // 2 + 2],
                                      ps[:, IN:IN + 2])
                nc.vector.tensor_copy(adL1[:, t, :], ps[:, IN + 2:IN + 4])
                nc.sync.dma_start(cc1_in[t * P:(t + 1) * P, 0:IN + 4],
                                  row[:])

        nc.gpsimd.collective_compute(
            "AllGather", ALU.bypass, replica_groups=[list(range(W))],
            ins=[cc1_in[:]], outs=[cc1_out[:]])

        gq = [0]  # global Pool-DMA emission counter: sem lane = gq%8,
        # so queue gq%4 keeps each DMASW sem pinned to one SWDGE queue

        # ---- per-edge a_dst staging (filled by pre-pass in AG windows)
        ad_st1 = consts.tile([P, NBLK, CPB, 2], f32)
        ad_st2 = consts.tile([P, NBLK, CPB, 1], f32)

        # shared SBUF pools for both layers (avoids an SBUF reallocation
        # barrier at phase boundaries that would stall the AG windows)
        pi = stack.enter_context(tc.tile_pool(name="idx", bufs=3))
        pg = stack.enter_context(tc.tile_pool(name="gath", bufs=4))
        pe_ = stack.enter_context(tc.tile_pool(name="ew", bufs=3))
        pm = stack.enter_context(tc.tile_pool(name="msg", bufs=3))
        po = stack.enter_context(tc.tile_pool(name="epi", bufs=2))
        pA = stack.enter_context(tc.tile_pool(name="pA", bufs=3))
        # init gather buffers once: stale data must stay finite
        for ig in range(4):
            g0 = pg.tile([P, CPB, ROWW], f16, tag="gt")
            nc.vector.memset(g0[:], 0.0)

        # ---------------- pre-pass: S2^T @ adb -> staged a_dst ----------
        def phase_a(layer):
            adL, adst, nhead = ((adL1, ad_st1, 2) if layer == 1
                                else (adL2, ad_st2, 1))
            with tc.tile_pool(name=f"pAp{layer}", bufs=2,
                              space="PSUM") as pAp:
                for blk in range(NBLK):
                    S2 = pA.tile([P, CPB, P], f8, tag="S2")
                    nc.sync.dma_start(
                        S2[:], s2hd[:, blk * CPB * P:(blk + 1) * CPB * P]
                        .rearrange("p (a b) -> p a b", a=CPB))
                    ad_ps = pAp.tile([P, CPB * nhead], f32, tag="adp")
                    for j in range(CPB):
                        nc.tensor.matmul(
                            ad_ps[:, j * nhead:(j + 1) * nhead],
                            lhsT=S2[:, j, :], rhs=adL[:, blk, :],
                            start=True, stop=True)
                    nc.vector.tensor_copy(
                        adst[:, blk].rearrange("p a b -> p (a b)"), ad_ps[:])

        # ---------------- main pass ------------------------------------
        def edge_phase(layer):
            if layer == 1:
                table, self_tab = cc1_out, cc1_in
                nhead, MSGC, adst, b_sb = 2, HID + 2, ad_st1, b1_sb
            else:
                table, self_tab = cc2_out, cc2_in
                nhead, MSGC, adst, b_sb = 1, OUT + 1, ad_st2, b2_sb
            tab_lo = table[0:LOSPLIT, :]
            tab_hi = table[LOSPLIT:NROWS, :]

            es = ExitStack()
            pp = es.enter_context(
                tc.tile_pool(name=f"bps{layer}", bufs=2, space="PSUM"))
            ph = es.enter_context(
                tc.tile_pool(name=f"hps{layer}", bufs=2, space="PSUM"))
            pn = es.enter_context(
                tc.tile_pool(name=f"nps{layer}", bufs=2, space="PSUM"))

            # S one-hots built one block ahead of consumption
            S_t = {}

            def prep_S(b):
                S = pm.tile([P, CPB, P], f8, tag="S")
                nc.sync.dma_start(
                    S[:], shd[:, b * CPB * P:(b + 1) * CPB * P]
                    .rearrange("p (a b) -> p a b", a=CPB))
                S_t[b] = S

            prep_S(0)
            for blk in range(NBLK):
                isrc = pi.tile([P, CPG * 8], i16, tag="isrc")
                nc.sync.dma_start(isrc[:],
                                  isrd[:, blk * CPG * 8:(blk + 1) * CPG * 8])

                gt = pg.tile([P, CPB, ROWW], f16, tag="gt")
                for c0 in range(0, CPL, 8):
                    c1 = min(c0 + 8, CPL)
                    nc.gpsimd.dma_gather(
                        gt[:, c0:c1, :], tab_lo,
                        isrc[:, c0 * 8:c1 * 8],
                        (c1 - c0) * P, (c1 - c0) * P, ROWW,
                        queue_num=gq[0] % 4)
                    gq[0] += 1
                for c0 in range(CPL, CPG, 8):
                    c1 = min(c0 + 8, CPG)
                    nc.gpsimd.dma_gather(
                        gt[:, c0:c1, :], tab_hi,
                        isrc[:, c0 * 8:c1 * 8],
                        (c1 - c0) * P, (c1 - c0) * P, ROWW,
                        queue_num=gq[0] % 4)
                    gq[0] += 1
                # self chunk: contiguous rows of own shard
                nc.sync.dma_start(gt[:, CPG, 0:IN + 2 * nhead],
                                  self_tab[blk * P:(blk + 1) * P,
                                           0:IN + 2 * nhead])

                if blk + 1 < NBLK:
                    prep_S(blk + 1)
                S_all = S_t.pop(blk)

                # ew = exp(lrelu(a_s + a_d, 0.2))
                gtf = gt[:].bitcast(f32)
                as_v = gtf[:, :, IN // 2:IN // 2 + nhead]
                ew_t = pe_.tile([P, CPB, 2], f32, tag="ew")
                ew = ew_t[:, :, 0:nhead]
                nc.vector.tensor_tensor(ew, as_v, adst[:, blk],
                                        op=ALU.add)
                ew2_t = pe_.tile([P, CPB, 2], f32, tag="ew2")
                ew2 = ew2_t[:, :, 0:nhead]
                nc.vector.tensor_tensor(
                    ew2, ew,
                    c02[:].unsqueeze(2).to_broadcast([P, CPB, nhead]),
                    op=ALU.mult)
                nc.vector.tensor_tensor(ew, ew2, ew, op=ALU.max)
                wv_t = pe_.tile([P, CPB, 2], bf16, tag="wv")
                wv = wv_t[:, :, 0:nhead]
                nc.scalar.activation(wv, ew, AF.Exp)

                # msg = [w*h | w], two halves so bp matmuls start early
                msg_t = pm.tile([P, CPB, HID + 2], bf16, tag="msg")
                msg = msg_t[:, :, 0:MSGC]
                bp_t = pp.tile([P, HID + 2], f32, tag="bp")
                bp = bp_t[:, 0:MSGC]
                HSPL = CPB // 2
                for h0, h1 in ((0, HSPL), (HSPL, CPB)):
                    nc.vector.tensor_tensor(
                        msg[:, h0:h1, 0:IN].rearrange(
                            "p a (h f) -> p a h f", h=nhead),
                        gt[:, h0:h1, 0:IN].rearrange(
                            "p a (h f) -> p a h f", h=nhead),
                        wv[:, h0:h1, :].unsqueeze(3).to_broadcast(
                            [P, h1 - h0, nhead, IN // nhead]),
                        op=ALU.mult)
                    nc.vector.tensor_copy(msg[:, h0:h1, IN:IN + nhead],
                                          wv[:, h0:h1, :])
                    for j in range(h0, h1):
                        nc.tensor.matmul(bp, lhsT=S_all[:, j, :],
                                         rhs=msg[:, j, :],
                                         start=(j == 0), stop=(j == CPB - 1))
                # ---- epilogue
                rec = po.tile([P, nhead], f32, tag="rec")
                nc.vector.reciprocal(rec[:], bp[:, IN:IN + nhead])
                ti = po.tile([P, IN], f32, tag="ti")
                nc.vector.tensor_tensor(
                    ti[:].rearrange("p (h f) -> p h f", h=nhead),
                    bp[:, 0:IN].rearrange("p (h f) -> p h f", h=nhead),
                    rec[:].unsqueeze(2).to_broadcast(
                        [P, nhead, IN // nhead]),
                    op=ALU.mult)
                nc.vector.tensor_tensor(ti[:], ti[:], b_sb[:], op=ALU.add)
                tif = po.tile([P, IN], f16, tag="tif")
                if layer == 1:
                    # leaky_relu 0.01 then cast
                    tl = po.tile([P, IN], f32, tag="tl")
                    nc.vector.tensor_tensor(
                        tl[:], ti[:], c001[:].to_broadcast([P, IN]),
                        op=ALU.mult)
                    nc.vector.tensor_tensor(tif[:], tl[:], ti[:], op=ALU.max)
                else:
                    nc.scalar.activation(tif[:], ti[:], AF.Copy)
                hT = po.tile([P, 2, P], f16, tag="hT")
                nc.sync.dma_start_transpose(hT[:], tif[:])

                if layer == 1:
                    hp = ph.tile([P, AUG2], f32, tag="hp")
                    for k in range(2):
                        nc.tensor.matmul(hp[:], lhsT=hT[:, k, :],
                                         rhs=w2_sb[:, k, :],
                                         start=(k == 0), stop=(k == 1))
                    row2 = po.tile([P, IN + 2], f16, tag="row2")
                    nc.scalar.activation(row2[:, 0:IN], hp[:, 0:IN], AF.Copy)
                    r2f = row2[:].bitcast(f32)
                    nc.vector.tensor_copy(r2f[:, IN // 2:IN // 2 + 1],
                                          hp[:, IN:IN + 1])
                    nc.scalar.activation(adL2[:, blk, :],
                                         hp[:, IN + 1:IN + 2], AF.Copy)
                    nc.sync.dma_start(cc2_in[blk * P:(blk + 1) * P, 0:IN + 2],
                                      row2[:])
                else:
                    # fused cosine head: out = h2 @ [g | gm]
                    op_ = ph.tile([P, KH * MD], f32, tag="op")
                    for k in range(2):
                        nc.tensor.matmul(op_[:, 0:512], lhsT=hT[:, k, :],
                                         rhs=g_sb[:, k, 0:512],
                                         start=(k == 0), stop=(k == 1))
                    for k in range(2):
                        nc.tensor.matmul(op_[:, 512:KH * MD],
                                         lhsT=hT[:, k, :],
                                         rhs=g_sb[:, k, 512:KH * MD],
                                         start=(k == 0), stop=(k == 1))
                    num_ps = pn.tile([P, KH], f32, tag="nm")
                    num_ps = num_ps[:]
                    for k in range(2):
                        nc.tensor.matmul(num_ps, lhsT=hT[:, k, :],
                                         rhs=g_sb[:, k, KH * MD:GAUG],
                                         start=(k == 0), stop=(k == 1))
                    for k in range(KH):
                        sqs = po.tile([P, MD], f16, tag="sqs")
                        nc.scalar.activation(
                            sqs[:], op_[:, k * MD:(k + 1) * MD], AF.Square,
                            accum_out=nrm2_st[:, blk, k:k + 1])
                    nc.scalar.activation(num_st[:, blk, :], num_ps, AF.Copy)

            es.close()

        phase_a(1)
        edge_phase(1)
        phase_a(2)
        nc.gpsimd.collective_compute(
            "AllGather", ALU.bypass, replica_groups=[list(range(W))],
            ins=[cc2_in[:]], outs=[cc2_out[:]])
        edge_phase(2)

        # ---------------- final cosine ----------------------------------
        with tc.tile_pool(name="fin", bufs=1) as fin:
            nrm = fin.tile([P, NBLK, KH], f32)
            nc.scalar.activation(nrm[:], nrm2_st[:], AF.Sqrt)
            nc.vector.tensor_tensor(
                nrm[:], nrm[:],
                cmu_sb[:].unsqueeze(1).to_broadcast([P, NBLK, KH]),
                op=ALU.mult)
            nc.vector.tensor_scalar(out=nrm[:], in0=nrm[:], scalar1=1e-8,
                                    scalar2=None, op0=ALU.max)
            rcp = fin.tile([P, NBLK, KH], f32)
            nc.vector.reciprocal(rcp[:], nrm[:])
            res = fin.tile([P, NBLK, KH], f32)
            nc.vector.tensor_tensor(res[:], num_st[:], rcp[:], op=ALU.mult)
            nc.sync.dma_start(
                outD[:].rearrange("(b p) k -> p b k", p=P), res[:])

    nc.compile()
    return nc


# ===================== host-side preparation ============================

def _wrap16(flat):
    """flat idx [n] -> wrapped int16 [128, n//16] (8 Q7-core replicas)."""
    n = len(flat)
    out = np.zeros((P, n // 16), np.int16)
    cols = np.arange(n) // 16
    rows = np.arange(n) % 16
    for r in range(8):
        out[r * 16 + rows, cols] = flat
    return out


def prep_host(x, edge_index, W1, a_src1, a_dst1, b1, W2, a_src2, a_dst2, b2,
              g, mu):
    x = np.asarray(x, np.float32)
    N = x.shape[0]
    src = np.asarray(edge_index[0], np.int64)
    dst = np.asarray(edge_index[1], np.int64)
    E = len(src)

    od = np.bincount(src, minlength=N)
    idg = np.bincount(dst, minlength=N)

    # node -> core: top-5/8 by out-degree to cores 0-4 (lo table half),
    # in-degree snake within each group for compute balance.
    PERCORE = N // W
    order_od = np.argsort(-od, kind="stable")
    groupA = order_od[:5 * PERCORE]
    groupB = order_od[5 * PERCORE:]
    core_of = np.empty(N, np.int32)
    for base, grp in ((0, groupA), (5, groupB)):
        ncg = len(grp) // PERCORE
        gs = grp[np.argsort(-idg[grp], kind="stable")]
        pat = np.concatenate([np.arange(ncg), np.arange(ncg)[::-1]])
        asn = np.tile(pat, (len(gs) + 2 * ncg - 1) // (2 * ncg))[:len(gs)]
        core_of[gs] = base + asn

    # per-edge lo flag: src in cores 0-4
    src_lo = core_of[src] <= 4

    # per-node lo/hi in-degree
    idl = np.bincount(dst[src_lo], minlength=N)
    idh = idg - idl

    # per-core block packing under (lo, hi, count) caps
    loS = max(idl[core_of == c].sum() for c in range(W))
    hiS = max(idh[core_of == c].sum() for c in range(W))
    cands = []
    for tot in range(16, 24):
        for cl in range(1, tot):
            ch = tot - cl
            if loS <= 0.975 * NBLK * cl * P and hiS <= 0.975 * NBLK * ch * P:
                cands.append((cl, ch))
    ci = 0
    while True:
        CPL, CPH = cands[ci]
        capL, capH = CPL * P, CPH * P
        blk_of = np.full(N, -1, np.int32)
        slot_of = np.full(N, -1, np.int32)
        ok = True
        for c in range(W):
            nodes = np.where(core_of == c)[0]
            nodes = nodes[np.argsort(-(idl[nodes] + idh[nodes]),
                                     kind="stable")]
            bl = np.zeros(NBLK, np.int64)
            bh = np.zeros(NBLK, np.int64)
            bn = np.zeros(NBLK, np.int64)
            for n in nodes:
                load = np.maximum((bl + idl[n]) / capL, (bh + idh[n]) / capH)
                load[bn >= P] = 10.0
                load[(bl + idl[n]) > capL] = 10.0
                load[(bh + idh[n]) > capH] = 10.0
                b = int(np.argmin(load))
                if load[b] >= 10.0:
                    ok = False
                    break
                blk_of[n] = b
                slot_of[n] = bn[b]
                bl[b] += idl[n]
                bh[b] += idh[n]
                bn[b] += 1
            if not ok:
                break
        if ok:
            break
        ci += 1

    cfg = CFG(N=N, CPL=CPL, CPH=CPH, GAUG=KH * MD + KH)
    CPG, CPB = cfg.CPG, cfg.CPB

    row_of = core_of.astype(np.int64) * CAP + blk_of * P + slot_of

    # group edges by (core, block), lo first then hi, sorted by src row
    gkey = core_of[dst].astype(np.int64) * NBLK + blk_of[dst]
    skey = gkey * 2 + (~src_lo)
    order = np.argsort(skey * NROWS + row_of[src], kind="stable")
    esrc_r = row_of[src][order]
    edst_l = slot_of[dst][order].astype(np.int64)
    eslo = src_lo[order]
    ekey = gkey[order]
    starts = np.zeros(W * NBLK + 1, np.int64)
    cnts = np.bincount(ekey, minlength=W * NBLK)
    starts[1:] = np.cumsum(cnts)
    lo_cnt = np.bincount(ekey[eslo], minlength=W * NBLK)

    import ml_dtypes
    isrc_all = []
    dstf_all = []
    s2h_all = []
    sh_all = []
    for c in range(W):
        isrc = np.full((P, NBLK * CPG * 8), -1, np.int16)
        dstf = np.full((P, NBLK * CPB), -1.0, np.float16)
        dint = np.full((P, NBLK * CPB), -1, np.int64)
        for b in range(NBLK):
            gid = c * NBLK + b
            s0, s1 = starts[gid], starts[gid + 1]
            nlo = int(lo_cnt[gid])
            rows_ = esrc_r[s0:s1]
            dl = edst_l[s0:s1]
            fl = np.zeros(CPG * P, np.int64)   # pad: fetch row 0 (valid)
            fd = np.full(CPG * P, -1, np.int64)
            fl[:nlo] = rows_[:nlo]
            fd[:nlo] = dl[:nlo]
            nh = (s1 - s0) - nlo
            fl[CPL * P:CPL * P + nh] = rows_[nlo:] - LOSPLIT
            fd[CPL * P:CPL * P + nh] = dl[nlo:]
            isrc[:, b * CPG * 8:(b + 1) * CPG * 8] = _wrap16(fl)
            dcol = fd.reshape(CPG, P).T.astype(np.float16)
            dstf[:, b * CPB:b * CPB + CPG] = dcol
            dstf[:, b * CPB + CPG] = np.arange(P, dtype=np.float16)
            dint[:, b * CPB:b * CPB + CPG] = fd.reshape(CPG, P).T
            dint[:, b * CPB + CPG] = np.arange(P)
        # transposed one-hot: S2h[d, (b j e)] = 1 iff dst(e-slot, b, j) == d
        s2 = np.zeros((P, NBLK * CPB * P), ml_dtypes.float8_e4m3)
        ee, bj = np.nonzero(dint >= 0)
        s2[dint[ee, bj], bj * P + ee] = 1
        sh = np.zeros((P, NBLK * CPB * P), ml_dtypes.float8_e4m3)
        sh[ee, bj * P + dint[ee, bj]] = 1
        isrc_all.append(isrc)
        dstf_all.append(dstf)
        s2h_all.append(s2)
        sh_all.append(sh)

    # weights
    W1 = np.asarray(W1, np.float32)
    W2 = np.asarray(W2, np.float32)
    W1r = W1.reshape(H1, MD, IN)
    Ps1 = np.einsum("hdi,hd->ih", W1r, np.asarray(a_src1, np.float32))
    Pd1 = np.einsum("hdi,hd->ih", W1r, np.asarray(a_dst1, np.float32))
    W1aug = np.concatenate([W1.T, Ps1, Pd1], axis=1)
    Ps2 = W2.T @ np.asarray(a_src2, np.float32)[0][:, None]
    Pd2 = W2.T @ np.asarray(a_dst2, np.float32)[0][:, None]
    W2aug = np.concatenate([W2.T, Ps2, Pd2], axis=1)
    AUG1, AUG2 = IN + 4, IN + 2
    w1s = W1aug.reshape(2, P, AUG1).transpose(1, 0, 2).astype(np.float16)
    w2s = W2aug.reshape(2, P, AUG2).transpose(1, 0, 2).astype(np.float16)

    gm = np.asarray(g, np.float32)
    mu = np.asarray(mu, np.float32)
    gmu = np.einsum("fkm,km->fk", gm.reshape(IN, KH, MD), mu)  # [256, 8]
    g_aug = np.concatenate([gm, gmu], axis=1)                  # [256, 1032]
    gsd = g_aug.reshape(2, P, cfg.GAUG).transpose(1, 0, 2).astype(np.float16)
    cmu = np.broadcast_to(np.linalg.norm(mu, axis=1), (P, KH)).astype(
        np.float32).copy()
    b1b = np.broadcast_to(np.asarray(b1, np.float32), (P, HID)).copy()
    b2b = np.broadcast_to(np.asarray(b2, np.float32), (P, OUT)).copy()
    iota = np.broadcast_to(np.arange(P, dtype=np.float16), (P, P)).copy()

    shared = dict(w1s=w1s, w2s=w2s, gs=gsd, cmu=cmu, b1b=b1b, b2b=b2b,
                  iota=iota)
    in_maps = []
    for c in range(W):
        nodes = np.where(core_of == c)[0]
        xp = np.zeros((CAP, IN), np.float32)
        xp[blk_of[nodes] * P + slot_of[nodes]] = x[nodes]
        xTc = xp.reshape(NBLK, P, 2, P).transpose(3, 0, 2, 1).astype(
            np.float16)
        m = dict(shared)
        m.update(xTi=xTc, isrc=isrc_all[c], dstf=dstf_all[c],
                 s2h=s2h_all[c], sh=sh_all[c])
        in_maps.append(m)
    return cfg, in_maps, row_of


def assemble(outs, row_of, N):
    full = np.zeros((N, KH), np.float32)
    core = row_of // CAP
    rrow = row_of % CAP
    for c in range(W):
        sel = core == c
        full[sel] = outs[c]["outD"][rrow[sel]]
    return full


_CACHE = {}


def kernel(**inputs):
    cfg, in_maps, row_of = prep_host(**inputs)
    key = (cfg.N, cfg.CPL, cfg.CPH)
    if key not in _CACHE:
        _CACHE[key] = build_program(cfg)
    nc = _CACHE[key]
    from concourse.bass_utils import run_bass_kernel_spmd
    res = run_bass_kernel_spmd(nc, in_maps, core_ids=list(range(W)))
    return assemble(res.results, row_of, cfg.N)
